# revision 17
# baseline (speedup 1.0000x reference)
"""AFNO2D Trainium2 kernel (8 NeuronCores, channel-sharded, zero collectives).

Each core processes one 96-channel block (FFT is per-channel; the MLP is
block-diagonal with exactly 8 blocks of 96 -> core i owns block i end-to-end).

Per-core pipeline (all matmuls bf16, fp32 PSUM). Layouts are chosen so that
every matmul rhs is contiguous-innermost and every PSUM eviction writes
contiguous (or long-run) destinations; evictions are split across DVE+ACT.

  S1  rfft over W:   lhsT=F1 [w,128]=[cos(65)|-sin(63)], rhs=xb [w, c, h]
                     4c-chunks -> psum [kwstack,(4c,128h)] -> t1 [kw, c, h]
  T1  DMA-xbar transpose (split on 2 engines): t1 -> t2 [h, c, kwstack]
  S2  DFT over H (data stationary): per kw: lhsT=t2[:, :, kw*] [h, c],
                     rhs=F2a/F2b [h, 256] -> psum [c, (khr|khi)] -> specw
  MLP1 (rhs mode):   lhsT=w1* [96,97] (col 96 zero, b1 row 96 = 1 ->
                     o1 row 96 == 1, the fused bias row for MLP2),
                     relu+b1 evict -> o1w [97, ri, kw, kh]
  MLP2 (data stationary): per kw: lhsT=o1w slices [97, kh],
                     rhs=[w2r|w2i ; b2r|b2i] / [-w2i|w2r ; 0]
                     -> psum [kh, (kw, cr|ci)]
  softshrink:        relu(v-l) + min(v+l, 0) -> o2 [kh, ri, c, kw]
  S4  iDFT over H (rhs mode): lhsT=Gc/Gs/-Gs, rhs=o2 4c-chunks (65-el runs)
                     -> psum [h, (4c, kw)] -> ubuf [h, c, kwstack]
                     (ui bins 0,64 dropped)
  T3  DMA-xbar transpose (split): ubuf -> s5rhs [kwstack, c, h]
  S5  irfft over W:  lhsT=Ainv [kstack, w], rhs=s5rhs + t1 (residual in
                     spectrum: irfft_W(rfft_W(x)) == x), 4c-chunks
                     -> psum [w, (4c, 128h)] -> out [w, c, h] f32

Host passes x pre-transposed to [B, W, 96, H] and un-transposes the
[B, W, 96, H] output, so all DMAs are fully contiguous per partition.
"""
import numpy as np
import ml_dtypes

B, H, W, C = 4, 128, 128, 768
NB, BL = 8, 96
WF = 65
LAMBD = 0.01
BF16 = ml_dtypes.bfloat16

_CACHE = {}


def _make_consts():
    w = np.arange(W, dtype=np.float64)[:, None]
    k = np.arange(WF, dtype=np.float64)[None, :]
    th = 2 * np.pi * w * k / W
    s = 1.0 / np.sqrt(W)
    f1 = np.concatenate([np.cos(th) * s, -np.sin(th[:, 1:64]) * s], axis=1)

    h = np.arange(H, dtype=np.float64)[:, None]
    kh = np.arange(H, dtype=np.float64)[None, :]
    th2 = 2 * np.pi * h * kh / H
    c2 = np.cos(th2) / np.sqrt(H)
    s2 = np.sin(th2) / np.sqrt(H)
    f2a = np.concatenate([c2, -s2], axis=1)   # rhs when lhsT = t_r
    f2b = np.concatenate([s2, c2], axis=1)    # rhs when lhsT = t_i

    gc = (np.cos(th2) / np.sqrt(H)).T         # [kh, h]
    gs = (np.sin(th2) / np.sqrt(H)).T

    kk = np.arange(WF, dtype=np.float64)[:, None]
    ww = np.arange(W, dtype=np.float64)[None, :]
    th3 = 2 * np.pi * kk * ww / W
    beta = np.full((WF, 1), 2.0); beta[0] = 1.0; beta[64] = 1.0
    ac = beta * np.cos(th3) / np.sqrt(W)
    asn = -2.0 * np.sin(th3[1:64]) / np.sqrt(W)
    ainv = np.concatenate([ac, asn], axis=0)

    cast = lambda a: np.ascontiguousarray(a).astype(BF16)
    return dict(f1=cast(f1), f2a=cast(f2a), f2b=cast(f2b),
                gc=cast(gc), gs=cast(gs), gsn=cast(-gs), ainv=cast(ainv))


def _groups():
    gs = [list(range(i, i + 8)) for i in range(0, 64, 8)]
    gs.append([64])
    return gs


def _build():
    from contextlib import ExitStack
    from concourse import bacc, mybir, tile

    dt = mybir.dt
    nc = bacc.Bacc("TRN2", target_bir_lowering=False, debug=False, num_devices=8)

    x_d = nc.dram_tensor("x", [B, W, BL, H], dt.bfloat16, kind="ExternalInput")
    f1_d = nc.dram_tensor("f1", [128, 128], dt.bfloat16, kind="ExternalInput")
    f2a_d = nc.dram_tensor("f2a", [128, 256], dt.bfloat16, kind="ExternalInput")
    f2b_d = nc.dram_tensor("f2b", [128, 256], dt.bfloat16, kind="ExternalInput")
    gc_d = nc.dram_tensor("gc", [128, 128], dt.bfloat16, kind="ExternalInput")
    gs_d = nc.dram_tensor("gs", [128, 128], dt.bfloat16, kind="ExternalInput")
    gsn_d = nc.dram_tensor("gsn", [128, 128], dt.bfloat16, kind="ExternalInput")
    ainv_d = nc.dram_tensor("ainv", [128, 128], dt.bfloat16, kind="ExternalInput")
    w1r_d = nc.dram_tensor("w1r", [BL, 97], dt.bfloat16, kind="ExternalInput")
    w1i_d = nc.dram_tensor("w1i", [BL, 97], dt.bfloat16, kind="ExternalInput")
    w1in_d = nc.dram_tensor("w1in", [BL, 97], dt.bfloat16, kind="ExternalInput")
    w2a_d = nc.dram_tensor("w2a", [97, 192], dt.bfloat16, kind="ExternalInput")
    w2b_d = nc.dram_tensor("w2b", [97, 192], dt.bfloat16, kind="ExternalInput")
    b1r_d = nc.dram_tensor("b1r", [97, 1], dt.float32, kind="ExternalInput")
    b1i_d = nc.dram_tensor("b1i", [97, 1], dt.float32, kind="ExternalInput")
    out_d = nc.dram_tensor("out", [B, W, BL, H], dt.float32, kind="ExternalOutput")

    Relu = mybir.ActivationFunctionType.Relu
    Ident = mybir.ActivationFunctionType.Identity
    ADD = mybir.AluOpType.add
    MAX = mybir.AluOpType.max
    MIN = mybir.AluOpType.min

    with tile.TileContext(nc) as tc, ExitStack() as ctx:
        cp = ctx.enter_context(tc.tile_pool(name="const", bufs=1))
        xp = ctx.enter_context(tc.tile_pool(name="xb", bufs=1))
        t1p = ctx.enter_context(tc.tile_pool(name="t1", bufs=2))
        t2p = ctx.enter_context(tc.tile_pool(name="t2", bufs=1))
        sw = ctx.enter_context(tc.tile_pool(name="specw", bufs=2))
        o1p = ctx.enter_context(tc.tile_pool(name="o1w", bufs=2))
        o2p = ctx.enter_context(tc.tile_pool(name="o2w", bufs=1))
        tap = ctx.enter_context(tc.tile_pool(name="tmpa", bufs=2))
        tbp = ctx.enter_context(tc.tile_pool(name="tmpb", bufs=2))
        up = ctx.enter_context(tc.tile_pool(name="ubuf", bufs=1))
        s5p = ctx.enter_context(tc.tile_pool(name="s5rhs", bufs=1))
        ocp = ctx.enter_context(tc.tile_pool(name="outc", bufs=2))
        psm = ctx.enter_context(tc.tile_pool(name="psmain", bufs=4, space="PSUM"))
        ps2p = ctx.enter_context(tc.tile_pool(name="ps2", bufs=2, space="PSUM"))
        pm2p = ctx.enter_context(tc.tile_pool(name="psm2", bufs=2, space="PSUM"))

        def cload(dram, shape, dtype=dt.bfloat16):
            t = cp.tile(shape, dtype, tag=f"c_{dram.name}")
            nc.sync.dma_start(t[:], dram[:])
            return t

        f1 = cload(f1_d, [128, 128]); f2a = cload(f2a_d, [128, 256])
        f2b = cload(f2b_d, [128, 256]); gc = cload(gc_d, [128, 128])
        gs = cload(gs_d, [128, 128]); gsn = cload(gsn_d, [128, 128])
        ainv = cload(ainv_d, [128, 128])
        w1r = cload(w1r_d, [BL, 97]); w1i = cload(w1i_d, [BL, 97])
        w1in = cload(w1in_d, [BL, 97])
        w2a = cload(w2a_d, [97, 192]); w2b = cload(w2b_d, [97, 192])
        b1r = cload(b1r_d, [97, 1], dt.float32)
        b1i = cload(b1i_d, [97, 1], dt.float32)
        lamneg = cp.tile([128, 1], dt.float32, tag="c_lamneg")
        nc.gpsimd.memset(lamneg[:], -LAMBD)
        zbias = cp.tile([128, 1], dt.float32, tag="c_zbias")
        nc.gpsimd.memset(zbias[:], 0.0)

        GROUPS = _groups()

        _flip = [0]

        def split_evict(dst_dve, src_dve, dst_act, src_act, act_bias=None,
                        act_func=None):
            # whole-tile eviction, alternating engines 3:2 DVE:ACT (two
            # engines reading halves of one PSUM tile raced on hardware)
            _flip[0] ^= 1
            if _flip[0]:
                nc.vector.tensor_copy(dst_dve, src_dve)
                nc.vector.tensor_copy(dst_act, src_act)
            else:
                p = src_dve.shape[0]
                nc.scalar.activation(dst_dve, src_dve, Ident, bias=zbias[0:p, :])
                nc.scalar.activation(dst_act, src_act, Ident, bias=zbias[0:p, :])

        for b in range(B):
            xb = xp.tile([128, BL, 128], dt.bfloat16, tag="xb")  # [w, c, h]
            nc.sync.dma_start(xb[:], x_d[b])

            # ---- S1: 4c chunks, contiguous rhs + contiguous split eviction
            t1 = t1p.tile([128, BL, 128], dt.bfloat16, tag="t1")  # [kw, c, h]
            for ci in range(0, BL, 4):
                ps = psm.tile([128, 4, 128], dt.float32, tag="ps")
                nc.tensor.matmul(ps[:], f1[:], xb[:, ci:ci + 4, :],
                                 start=True, stop=True)
                split_evict(t1[:, ci:ci + 2, :], ps[:, 0:2, :],
                            t1[:, ci + 2:ci + 4, :], ps[:, 2:4, :])

            # ---- T1 (split across the two hwdge engines)
            t2 = t2p.tile([128, BL, 128], dt.bfloat16, tag="t2")  # [h, c, kwstack]
            for qi, q in enumerate(range(0, BL, 24)):
                eng = nc.sync if qi % 2 == 0 else nc.scalar
                eng.dma_start_transpose(t2[:, q:q + 24, :], t1[:, q:q + 24, :])

            # ---- middle section per kw-group
            ub = up.tile([128, BL, 128], dt.bfloat16, tag="ub")  # [h, c, kwstack]
            o2 = o2p.tile([128, 2, BL, WF], dt.bfloat16, tag="o2")  # [kh,ri,c,kw]
            for grp in GROUPS:
                g0, gl = grp[0], len(grp)
                spec = sw.tile([BL, 8, 2, 128], dt.bfloat16, tag="spec")
                # S2: two kw per psum tile; split eviction (khr -> DVE, khi -> ACT)
                for j0 in range(0, gl, 2):
                    jl = min(2, gl - j0)
                    ps2 = ps2p.tile([BL, 2, 2, 128], dt.float32, tag="ps2")
                    for j in range(j0, j0 + jl):
                        kw = g0 + j
                        edge = kw in (0, 64)
                        nc.tensor.matmul(ps2[:, j - j0, :, :], t2[:, :, kw],
                                         f2a[:], start=True, stop=edge)
                        if not edge:
                            nc.tensor.matmul(ps2[:, j - j0, :, :],
                                             t2[:, :, 64 + kw], f2b[:],
                                             start=False, stop=True)
                    split_evict(spec[:, j0:j0 + jl, 0, :], ps2[:, 0:jl, 0, :],
                                spec[:, j0:j0 + jl, 1, :], ps2[:, 0:jl, 1, :])
                # MLP1 over this window (M=97: row 96 == 1.0 bias row)
                o1 = o1p.tile([97, 2, 8, 128], dt.bfloat16, tag="o1")
                for c0 in range(0, gl, 4):
                    cl = min(4, gl - c0)
                    xr = spec[:, c0:c0 + cl, 0, :]
                    xi = spec[:, c0:c0 + cl, 1, :]
                    pr = psm.tile([97, 4, 128], dt.float32, tag="ps")
                    nc.tensor.matmul(pr[:, 0:cl, :], w1r[:], xr, start=True, stop=False)
                    nc.tensor.matmul(pr[:, 0:cl, :], w1in[:], xi, start=False, stop=True)
                    pi = psm.tile([97, 4, 128], dt.float32, tag="ps")
                    nc.tensor.matmul(pi[:, 0:cl, :], w1i[:], xr, start=True, stop=False)
                    nc.tensor.matmul(pi[:, 0:cl, :], w1r[:], xi, start=False, stop=True)
                    nc.scalar.activation(o1[:, 0, c0:c0 + cl, :],
                                         pr[:, 0:cl, :], Relu, bias=b1r[:])
                    nc.scalar.activation(o1[:, 1, c0:c0 + cl, :],
                                         pi[:, 0:cl, :], Relu, bias=b1i[:])
                # MLP2 (bias via o1 ones-row) + softshrink -> o2 [kh, ri, c, kw]
                for j0 in range(0, gl, 2):
                    jl = min(2, gl - j0)
                    pm = pm2p.tile([128, 2, 2, BL], dt.float32, tag="pm2")
                    for j in range(j0, j0 + jl):
                        nc.tensor.matmul(pm[:, j - j0, :, :], o1[:, 0, j, :],
                                         w2a[:], start=True, stop=False)
                        nc.tensor.matmul(pm[:, j - j0, :, :], o1[:, 1, j, :],
                                         w2b[:], start=False, stop=True)
                    # softshrink(v) = relu(v - l) + min(v + l, 0)
                    ta = tap.tile([128, 2, 2, BL], dt.bfloat16, tag="ta")
                    tb = tbp.tile([128, 2, 2, BL], dt.bfloat16, tag="tb")
                    nc.scalar.activation(ta[:, 0:jl], pm[:, 0:jl], Relu,
                                         bias=lamneg[:])
                    nc.vector.tensor_scalar(tb[:, 0:jl], pm[:, 0:jl],
                                            LAMBD, 0.0, ADD, MIN)
                    dst = o2[:, :, :, g0 + j0:g0 + j0 + jl].rearrange(
                        "k r c w -> k w r c")
                    _flip[0] ^= 1
                    eng = nc.gpsimd if (g0 + j0) % 6 else nc.vector
                    eng.tensor_tensor(dst, ta[:, 0:jl], tb[:, 0:jl], ADD)

            # ---- S4: 4c chunks; rhs 65-el runs; evictions 65/63-el runs, split
            for ci in range(0, BL, 4):
                o2r = o2[:, 0, ci:ci + 4, :]
                o2i = o2[:, 1, ci:ci + 4, :]
                pu = psm.tile([128, 4, WF], dt.float32, tag="ps")
                nc.tensor.matmul(pu[:], gc[:], o2r, start=True, stop=False)
                nc.tensor.matmul(pu[:], gsn[:], o2i, start=False, stop=True)
                split_evict(ub[:, ci:ci + 2, 0:WF], pu[:, 0:2, :],
                            ub[:, ci + 2:ci + 4, 0:WF], pu[:, 2:4, :])
                pv = psm.tile([128, 4, 64], dt.float32, tag="ps")
                nc.tensor.matmul(pv[:], gs[:], o2r[:, :, 0:64],
                                 start=True, stop=False)
                nc.tensor.matmul(pv[:], gc[:], o2i[:, :, 0:64],
                                 start=False, stop=True)
                split_evict(ub[:, ci:ci + 2, WF:128], pv[:, 0:2, 1:64],
                            ub[:, ci + 2:ci + 4, WF:128], pv[:, 2:4, 1:64])

            # ---- T3 (split)
            s5r = s5p.tile([128, BL, 128], dt.bfloat16, tag="s5r")  # [kstack, c, h]
            for qi, q in enumerate(range(0, BL, 24)):
                eng = nc.sync if qi % 2 == 0 else nc.scalar
                eng.dma_start_transpose(s5r[:, q:q + 24, :], ub[:, q:q + 24, :])

            # ---- S5 with residual-in-spectrum; contiguous; batched out-DMA
            for co in range(0, BL, 12):
                oc = ocp.tile([128, 12, 128], dt.float32, tag="oc")  # [w, c, h]
                for ci in range(co, co + 12, 4):
                    ps5 = psm.tile([128, 4, 128], dt.float32, tag="ps")
                    nc.tensor.matmul(ps5[:], ainv[:], s5r[:, ci:ci + 4, :],
                                     start=True, stop=False)
                    nc.tensor.matmul(ps5[:], ainv[:], t1[:, ci:ci + 4, :],
                                     start=False, stop=True)
                    split_evict(oc[:, ci - co:ci - co + 2, :], ps5[:, 0:2, :],
                                oc[:, ci - co + 2:ci - co + 4, :], ps5[:, 2:4, :])
                nc.sync.dma_start(out_d[b, :, co:co + 12, :], oc[:])

    nc.compile()
    return nc


def get_nc():
    if "nc" not in _CACHE:
        _CACHE["nc"] = _build()
    return _CACHE["nc"]


def make_in_maps(x, w1, b1, w2, b2):
    consts = _make_consts()
    f32 = np.float32
    in_maps = []
    for i in range(NB):
        sl = slice(BL * i, BL * (i + 1))
        m = dict(consts)
        m["x"] = np.ascontiguousarray(
            x[..., sl].transpose(0, 2, 3, 1)).astype(BF16)  # [B, W, 96, H]
        w1p = np.zeros((2, BL, 97), np.float64)
        w1p[:, :, 0:BL] = w1[:, i]
        m["w1r"] = w1p[0].astype(BF16)
        m["w1i"] = w1p[1].astype(BF16)
        m["w1in"] = (-w1p[1]).astype(BF16)
        w2a = np.zeros((97, 192), np.float64)
        w2a[0:BL, 0:BL] = w2[0, i]; w2a[0:BL, BL:] = w2[1, i]
        w2a[BL, 0:BL] = b2[0, i]; w2a[BL, BL:] = b2[1, i]
        w2bp = np.zeros((97, 192), np.float64)
        w2bp[0:BL, 0:BL] = -w2[1, i]; w2bp[0:BL, BL:] = w2[0, i]
        m["w2a"] = w2a.astype(BF16)
        m["w2b"] = w2bp.astype(BF16)
        b1p = np.zeros((2, 97, 1), np.float64)
        b1p[:, 0:BL, 0] = b1[:, i]
        b1p[0, BL, 0] = 1.0  # ones row for fused MLP2 bias
        m["b1r"] = b1p[0].astype(f32)
        m["b1i"] = b1p[1].astype(f32)
        in_maps.append(m)
    return in_maps


def kernel(x, w1, b1, w2, b2):
    from concourse.bass_utils import run_bass_kernel_spmd
    nc = get_nc()
    in_maps = make_in_maps(np.asarray(x), np.asarray(w1), np.asarray(b1),
                           np.asarray(w2), np.asarray(b2))
    res = run_bass_kernel_spmd(nc, in_maps, core_ids=list(range(NB)))
    out = np.concatenate(
        [res.results[i]["out"].transpose(0, 3, 1, 2) for i in range(NB)],
        axis=-1)
    return out.astype(np.float32)


# revision 19
# speedup vs baseline: 1.0333x; 1.0333x over previous
"""AFNO2D Trainium2 kernel (8 NeuronCores, channel-sharded, zero collectives).

Each core processes one 96-channel block (FFT is per-channel; the MLP is
block-diagonal with exactly 8 blocks of 96 -> core i owns block i end-to-end).

Per-core pipeline (all matmuls bf16, fp32 PSUM). Layouts are chosen so that
every matmul rhs is contiguous-innermost and every PSUM eviction writes
contiguous (or long-run) destinations; evictions are split across DVE+ACT.

  S1  rfft over W:   lhsT=F1 [w,128]=[cos(65)|-sin(63)], rhs=xb [w, c, h]
                     4c-chunks -> psum [kwstack,(4c,128h)] -> t1 [kw, c, h]
  T1  DMA-xbar transpose (split on 2 engines): t1 -> t2 [h, c, kwstack]
  S2  DFT over H (data stationary): per kw: lhsT=t2[:, :, kw*] [h, c],
                     rhs=F2a/F2b [h, 256] -> psum [c, (khr|khi)] -> specw
  MLP1 (rhs mode):   lhsT=w1* [96,97] (col 96 zero, b1 row 96 = 1 ->
                     o1 row 96 == 1, the fused bias row for MLP2),
                     relu+b1 evict -> o1w [97, ri, kw, kh]
  MLP2 (data stationary): per kw: lhsT=o1w slices [97, kh],
                     rhs=[w2r|w2i ; b2r|b2i] / [-w2i|w2r ; 0]
                     -> psum [kh, (kw, cr|ci)]
  softshrink:        relu(v-l) + min(v+l, 0) -> o2 [kh, ri, c, kw]
  S4  iDFT over H (rhs mode): lhsT=Gc/Gs/-Gs, rhs=o2 4c-chunks (65-el runs)
                     -> psum [h, (4c, kw)] -> ubuf [h, c, kwstack]
                     (ui bins 0,64 dropped)
  T3  DMA-xbar transpose (split): ubuf -> s5rhs [kwstack, c, h]
  S5  irfft over W:  lhsT=Ainv [kstack, w], rhs=s5rhs + t1 (residual in
                     spectrum: irfft_W(rfft_W(x)) == x), 4c-chunks
                     -> psum [w, (4c, 128h)] -> out [w, c, h] f32

Host passes x pre-transposed to [B, W, 96, H] and un-transposes the
[B, W, 96, H] output, so all DMAs are fully contiguous per partition.
"""
import numpy as np
import ml_dtypes

B, H, W, C = 4, 128, 128, 768
NB, BL = 8, 96
WF = 65
LAMBD = 0.01
BF16 = ml_dtypes.bfloat16

_CACHE = {}


def _make_consts():
    w = np.arange(W, dtype=np.float64)[:, None]
    k = np.arange(WF, dtype=np.float64)[None, :]
    th = 2 * np.pi * w * k / W
    s = 1.0 / np.sqrt(W)
    f1 = np.concatenate([np.cos(th) * s, -np.sin(th[:, 1:64]) * s], axis=1)

    h = np.arange(H, dtype=np.float64)[:, None]
    kh = np.arange(H, dtype=np.float64)[None, :]
    th2 = 2 * np.pi * h * kh / H
    c2 = np.cos(th2) / np.sqrt(H)
    s2 = np.sin(th2) / np.sqrt(H)
    f2a = np.concatenate([c2, -s2], axis=1)   # rhs when lhsT = t_r
    f2b = np.concatenate([s2, c2], axis=1)    # rhs when lhsT = t_i

    gc = (np.cos(th2) / np.sqrt(H)).T         # [kh, h]
    gs = (np.sin(th2) / np.sqrt(H)).T

    kk = np.arange(WF, dtype=np.float64)[:, None]
    ww = np.arange(W, dtype=np.float64)[None, :]
    th3 = 2 * np.pi * kk * ww / W
    beta = np.full((WF, 1), 2.0); beta[0] = 1.0; beta[64] = 1.0
    ac = beta * np.cos(th3) / np.sqrt(W)
    asn = -2.0 * np.sin(th3[1:64]) / np.sqrt(W)
    ainv = np.concatenate([ac, asn], axis=0)

    cast = lambda a: np.ascontiguousarray(a).astype(BF16)
    return dict(f1=cast(f1), f2a=cast(f2a), f2b=cast(f2b),
                gc=cast(gc), gs=cast(gs), gsn=cast(-gs), ainv=cast(ainv))


def _groups():
    gs = [list(range(i, i + 8)) for i in range(0, 64, 8)]
    gs.append([64])
    return gs


def _build():
    from contextlib import ExitStack
    from concourse import bacc, mybir, tile

    dt = mybir.dt
    nc = bacc.Bacc("TRN2", target_bir_lowering=False, debug=False, num_devices=8)

    x_d = nc.dram_tensor("x", [B, W, BL, H], dt.bfloat16, kind="ExternalInput")
    f1_d = nc.dram_tensor("f1", [128, 128], dt.bfloat16, kind="ExternalInput")
    f2a_d = nc.dram_tensor("f2a", [128, 256], dt.bfloat16, kind="ExternalInput")
    f2b_d = nc.dram_tensor("f2b", [128, 256], dt.bfloat16, kind="ExternalInput")
    gc_d = nc.dram_tensor("gc", [128, 128], dt.bfloat16, kind="ExternalInput")
    gs_d = nc.dram_tensor("gs", [128, 128], dt.bfloat16, kind="ExternalInput")
    gsn_d = nc.dram_tensor("gsn", [128, 128], dt.bfloat16, kind="ExternalInput")
    ainv_d = nc.dram_tensor("ainv", [128, 128], dt.bfloat16, kind="ExternalInput")
    w1r_d = nc.dram_tensor("w1r", [BL, 97], dt.bfloat16, kind="ExternalInput")
    w1i_d = nc.dram_tensor("w1i", [BL, 97], dt.bfloat16, kind="ExternalInput")
    w1in_d = nc.dram_tensor("w1in", [BL, 97], dt.bfloat16, kind="ExternalInput")
    w2a_d = nc.dram_tensor("w2a", [97, 192], dt.bfloat16, kind="ExternalInput")
    w2b_d = nc.dram_tensor("w2b", [97, 192], dt.bfloat16, kind="ExternalInput")
    b1r_d = nc.dram_tensor("b1r", [97, 1], dt.float32, kind="ExternalInput")
    b1i_d = nc.dram_tensor("b1i", [97, 1], dt.float32, kind="ExternalInput")
    out_d = nc.dram_tensor("out", [B, W, BL, H], dt.float32, kind="ExternalOutput")

    Relu = mybir.ActivationFunctionType.Relu
    Ident = mybir.ActivationFunctionType.Identity
    ADD = mybir.AluOpType.add
    MAX = mybir.AluOpType.max
    MIN = mybir.AluOpType.min

    with tile.TileContext(nc) as tc, ExitStack() as ctx:
        cp = ctx.enter_context(tc.tile_pool(name="const", bufs=1))
        xp = ctx.enter_context(tc.tile_pool(name="xb", bufs=1))
        t1p = ctx.enter_context(tc.tile_pool(name="t1", bufs=2))
        t2p = ctx.enter_context(tc.tile_pool(name="t2", bufs=1))
        sw = ctx.enter_context(tc.tile_pool(name="specw", bufs=2))
        o1p = ctx.enter_context(tc.tile_pool(name="o1w", bufs=2))
        o2p = ctx.enter_context(tc.tile_pool(name="o2w", bufs=1))
        tap = ctx.enter_context(tc.tile_pool(name="tmpa", bufs=2))
        tbp = ctx.enter_context(tc.tile_pool(name="tmpb", bufs=2))
        up = ctx.enter_context(tc.tile_pool(name="ubuf", bufs=1))
        s5p = ctx.enter_context(tc.tile_pool(name="s5rhs", bufs=1))
        ocp = ctx.enter_context(tc.tile_pool(name="outc", bufs=2))
        psm = ctx.enter_context(tc.tile_pool(name="psmain", bufs=4, space="PSUM"))
        ps2p = ctx.enter_context(tc.tile_pool(name="ps2", bufs=2, space="PSUM"))
        pm2p = ctx.enter_context(tc.tile_pool(name="psm2", bufs=2, space="PSUM"))

        def cload(dram, shape, dtype=dt.bfloat16):
            t = cp.tile(shape, dtype, tag=f"c_{dram.name}")
            nc.sync.dma_start(t[:], dram[:])
            return t

        f1 = cload(f1_d, [128, 128]); f2a = cload(f2a_d, [128, 256])
        f2b = cload(f2b_d, [128, 256]); gc = cload(gc_d, [128, 128])
        gs = cload(gs_d, [128, 128]); gsn = cload(gsn_d, [128, 128])
        ainv = cload(ainv_d, [128, 128])
        w1r = cload(w1r_d, [BL, 97]); w1i = cload(w1i_d, [BL, 97])
        w1in = cload(w1in_d, [BL, 97])
        w2a = cload(w2a_d, [97, 192]); w2b = cload(w2b_d, [97, 192])
        b1r = cload(b1r_d, [97, 1], dt.float32)
        b1i = cload(b1i_d, [97, 1], dt.float32)
        lamneg = cp.tile([128, 1], dt.float32, tag="c_lamneg")
        nc.gpsimd.memset(lamneg[:], -LAMBD)
        zbias = cp.tile([128, 1], dt.float32, tag="c_zbias")
        nc.gpsimd.memset(zbias[:], 0.0)

        GROUPS = _groups()

        _flip = [0]

        def split_evict(dst_dve, src_dve, dst_act, src_act, act_bias=None,
                        act_func=None):
            # whole-tile eviction, alternating engines 3:2 DVE:ACT (two
            # engines reading halves of one PSUM tile raced on hardware)
            _flip[0] ^= 1
            if _flip[0]:
                nc.vector.tensor_copy(dst_dve, src_dve)
                nc.vector.tensor_copy(dst_act, src_act)
            else:
                p = src_dve.shape[0]
                nc.scalar.activation(dst_dve, src_dve, Ident, bias=zbias[0:p, :])
                nc.scalar.activation(dst_act, src_act, Ident, bias=zbias[0:p, :])

        prev = None  # (t1, s5r) of previous batch, for interleaved S5

        def emit_s5_group(st, gi):
            # 3 S5 chunks (12 c-columns) + 1 out-DMA per kw-group slot
            if st is None or gi >= 8:
                return
            t1q, s5q, bq = st
            co = gi * 12
            oc = ocp.tile([128, 12, 128], dt.float32, tag="oc")  # [w, c, h]
            for ci in range(co, co + 12, 4):
                ps5 = psm.tile([128, 4, 128], dt.float32, tag="ps")
                nc.tensor.matmul(ps5[:], ainv[:], s5q[:, ci:ci + 4, :],
                                 start=True, stop=False)
                nc.tensor.matmul(ps5[:], ainv[:], t1q[:, ci:ci + 4, :],
                                 start=False, stop=True)
                split_evict(oc[:, ci - co:ci - co + 2, :], ps5[:, 0:2, :],
                            oc[:, ci - co + 2:ci - co + 4, :], ps5[:, 2:4, :])
            nc.sync.dma_start(out_d[bq, :, co:co + 12, :], oc[:])

        for b in range(B + 1):
            if b == B:
                for gi in range(8):
                    emit_s5_group(prev, gi)
                break
            xb = xp.tile([128, BL, 128], dt.bfloat16, tag="xb")  # [w, c, h]
            nc.sync.dma_start(xb[:], x_d[b])

            # ---- S1: 4c chunks, contiguous rhs + contiguous eviction
            t1 = t1p.tile([128, BL, 128], dt.bfloat16, tag="t1")  # [kw, c, h]
            for ci in range(0, BL, 4):
                ps = psm.tile([128, 4, 128], dt.float32, tag="ps")
                nc.tensor.matmul(ps[:], f1[:], xb[:, ci:ci + 4, :],
                                 start=True, stop=True)
                split_evict(t1[:, ci:ci + 2, :], ps[:, 0:2, :],
                            t1[:, ci + 2:ci + 4, :], ps[:, 2:4, :])

            # ---- T1 (halves on the two hwdge engines)
            t2 = t2p.tile([128, BL, 128], dt.bfloat16, tag="t2")  # [h, c, kwstack]
            nc.sync.dma_start_transpose(t2[:, 0:48, :], t1[:, 0:48, :])
            nc.scalar.dma_start_transpose(t2[:, 48:BL, :], t1[:, 48:BL, :])

            # ---- middle section per kw-group, interleaved with S5(b-1)
            ub = up.tile([128, BL, 128], dt.bfloat16, tag="ub")  # [h, c, kwstack]
            o2 = o2p.tile([128, 2, BL, WF], dt.bfloat16, tag="o2")  # [kh,ri,c,kw]
            for gi, grp in enumerate(GROUPS):
                g0, gl = grp[0], len(grp)
                spec = sw.tile([BL, 8, 2, 128], dt.bfloat16, tag="spec")
                for j0 in range(0, gl, 2):
                    jl = min(2, gl - j0)
                    ps2 = ps2p.tile([BL, 2, 2, 128], dt.float32, tag="ps2")
                    for j in range(j0, j0 + jl):
                        kw = g0 + j
                        edge = kw in (0, 64)
                        nc.tensor.matmul(ps2[:, j - j0, :, :], t2[:, :, kw],
                                         f2a[:], start=True, stop=edge)
                        if not edge:
                            nc.tensor.matmul(ps2[:, j - j0, :, :],
                                             t2[:, :, 64 + kw], f2b[:],
                                             start=False, stop=True)
                    split_evict(spec[:, j0:j0 + jl, 0, :], ps2[:, 0:jl, 0, :],
                                spec[:, j0:j0 + jl, 1, :], ps2[:, 0:jl, 1, :])
                # MLP1 (M=97: row 96 == 1.0 bias row)
                o1 = o1p.tile([97, 2, 8, 128], dt.bfloat16, tag="o1")
                for c0 in range(0, gl, 4):
                    cl = min(4, gl - c0)
                    xr = spec[:, c0:c0 + cl, 0, :]
                    xi = spec[:, c0:c0 + cl, 1, :]
                    pr = psm.tile([97, 4, 128], dt.float32, tag="ps")
                    nc.tensor.matmul(pr[:, 0:cl, :], w1r[:], xr, start=True, stop=False)
                    nc.tensor.matmul(pr[:, 0:cl, :], w1in[:], xi, start=False, stop=True)
                    pi = psm.tile([97, 4, 128], dt.float32, tag="ps")
                    nc.tensor.matmul(pi[:, 0:cl, :], w1i[:], xr, start=True, stop=False)
                    nc.tensor.matmul(pi[:, 0:cl, :], w1r[:], xi, start=False, stop=True)
                    nc.scalar.activation(o1[:, 0, c0:c0 + cl, :], pr[:, 0:cl, :],
                                         Relu, bias=b1r[:])
                    nc.scalar.activation(o1[:, 1, c0:c0 + cl, :], pi[:, 0:cl, :],
                                         Relu, bias=b1i[:])
                # MLP2 (bias via o1 ones-row) + softshrink -> o2 [kh, ri, c, kw]
                for j0 in range(0, gl, 2):
                    jl = min(2, gl - j0)
                    pm = pm2p.tile([128, 2, 2, BL], dt.float32, tag="pm2")
                    for j in range(j0, j0 + jl):
                        nc.tensor.matmul(pm[:, j - j0, :, :], o1[:, 0, j, :],
                                         w2a[:], start=True, stop=False)
                        nc.tensor.matmul(pm[:, j - j0, :, :], o1[:, 1, j, :],
                                         w2b[:], start=False, stop=True)
                    ta = tap.tile([128, 2, 2, BL], dt.bfloat16, tag="ta")
                    tb = tbp.tile([128, 2, 2, BL], dt.bfloat16, tag="tb")
                    nc.scalar.activation(ta[:, 0:jl], pm[:, 0:jl], Relu,
                                         bias=lamneg[:])
                    nc.vector.tensor_scalar(tb[:, 0:jl], pm[:, 0:jl],
                                            LAMBD, 0.0, ADD, MIN)
                    dst = o2[:, :, :, g0 + j0:g0 + j0 + jl].rearrange(
                        "k r c w -> k w r c")
                    eng = nc.gpsimd if (g0 + j0) % 6 else nc.vector
                    eng.tensor_tensor(dst, ta[:, 0:jl], tb[:, 0:jl], ADD)
                # interleave previous batch's S5 work
                emit_s5_group(prev, gi)

            # ---- S4: 4c chunks; rhs 65-el runs; evictions 65/63-el runs
            for ci in range(0, BL, 4):
                o2r = o2[:, 0, ci:ci + 4, :]
                o2i = o2[:, 1, ci:ci + 4, :]
                pu = psm.tile([128, 4, WF], dt.float32, tag="ps")
                nc.tensor.matmul(pu[:], gc[:], o2r, start=True, stop=False)
                nc.tensor.matmul(pu[:], gsn[:], o2i, start=False, stop=True)
                split_evict(ub[:, ci:ci + 2, 0:WF], pu[:, 0:2, :],
                            ub[:, ci + 2:ci + 4, 0:WF], pu[:, 2:4, :])
                pv = psm.tile([128, 4, 64], dt.float32, tag="ps")
                nc.tensor.matmul(pv[:], gs[:], o2r[:, :, 0:64],
                                 start=True, stop=False)
                nc.tensor.matmul(pv[:], gc[:], o2i[:, :, 0:64],
                                 start=False, stop=True)
                split_evict(ub[:, ci:ci + 2, WF:128], pv[:, 0:2, 1:64],
                            ub[:, ci + 2:ci + 4, WF:128], pv[:, 2:4, 1:64])

            # ---- T3 (halves)
            s5r = s5p.tile([128, BL, 128], dt.bfloat16, tag="s5r")  # [kstack, c, h]
            nc.sync.dma_start_transpose(s5r[:, 0:48, :], ub[:, 0:48, :])
            nc.scalar.dma_start_transpose(s5r[:, 48:BL, :], ub[:, 48:BL, :])
            prev = (t1, s5r, b)

    nc.compile()
    return nc


def get_nc():
    if "nc" not in _CACHE:
        _CACHE["nc"] = _build()
    return _CACHE["nc"]


def make_in_maps(x, w1, b1, w2, b2):
    consts = _make_consts()
    f32 = np.float32
    in_maps = []
    for i in range(NB):
        sl = slice(BL * i, BL * (i + 1))
        m = dict(consts)
        m["x"] = np.ascontiguousarray(
            x[..., sl].transpose(0, 2, 3, 1)).astype(BF16)  # [B, W, 96, H]
        w1p = np.zeros((2, BL, 97), np.float64)
        w1p[:, :, 0:BL] = w1[:, i]
        m["w1r"] = w1p[0].astype(BF16)
        m["w1i"] = w1p[1].astype(BF16)
        m["w1in"] = (-w1p[1]).astype(BF16)
        w2a = np.zeros((97, 192), np.float64)
        w2a[0:BL, 0:BL] = w2[0, i]; w2a[0:BL, BL:] = w2[1, i]
        w2a[BL, 0:BL] = b2[0, i]; w2a[BL, BL:] = b2[1, i]
        w2bp = np.zeros((97, 192), np.float64)
        w2bp[0:BL, 0:BL] = -w2[1, i]; w2bp[0:BL, BL:] = w2[0, i]
        m["w2a"] = w2a.astype(BF16)
        m["w2b"] = w2bp.astype(BF16)
        b1p = np.zeros((2, 97, 1), np.float64)
        b1p[:, 0:BL, 0] = b1[:, i]
        b1p[0, BL, 0] = 1.0  # ones row for fused MLP2 bias
        m["b1r"] = b1p[0].astype(f32)
        m["b1i"] = b1p[1].astype(f32)
        in_maps.append(m)
    return in_maps


def kernel(x, w1, b1, w2, b2):
    from concourse.bass_utils import run_bass_kernel_spmd
    nc = get_nc()
    in_maps = make_in_maps(np.asarray(x), np.asarray(w1), np.asarray(b1),
                           np.asarray(w2), np.asarray(b2))
    res = run_bass_kernel_spmd(nc, in_maps, core_ids=list(range(NB)))
    out = np.concatenate(
        [res.results[i]["out"].transpose(0, 3, 1, 2) for i in range(NB)],
        axis=-1)
    return out.astype(np.float32)


# revision 20
# speedup vs baseline: 1.0527x; 1.0187x over previous
"""AFNO2D Trainium2 kernel (8 NeuronCores, channel-sharded, zero collectives).

Each core processes one 96-channel block (FFT is per-channel; the MLP is
block-diagonal with exactly 8 blocks of 96 -> core i owns block i end-to-end).

Per-core pipeline (all matmuls bf16, fp32 PSUM). Layouts are chosen so that
every matmul rhs is contiguous-innermost and every PSUM eviction writes
contiguous (or long-run) destinations; evictions are split across DVE+ACT.

  S1  rfft over W:   lhsT=F1 [w,128]=[cos(65)|-sin(63)], rhs=xb [w, c, h]
                     4c-chunks -> psum [kwstack,(4c,128h)] -> t1 [kw, c, h]
  T1  DMA-xbar transpose (split on 2 engines): t1 -> t2 [h, c, kwstack]
  S2  DFT over H (data stationary): per kw: lhsT=t2[:, :, kw*] [h, c],
                     rhs=F2a/F2b [h, 256] -> psum [c, (khr|khi)] -> specw
  MLP1 (rhs mode):   lhsT=w1* [96,97] (col 96 zero, b1 row 96 = 1 ->
                     o1 row 96 == 1, the fused bias row for MLP2),
                     relu+b1 evict -> o1w [97, ri, kw, kh]
  MLP2 (data stationary): per kw: lhsT=o1w slices [97, kh],
                     rhs=[w2r|w2i ; b2r|b2i] / [-w2i|w2r ; 0]
                     -> psum [kh, (kw, cr|ci)]
  softshrink:        relu(v-l) + min(v+l, 0) -> o2 [kh, ri, c, kw]
  S4  iDFT over H (rhs mode): lhsT=Gc/Gs/-Gs, rhs=o2 4c-chunks (65-el runs)
                     -> psum [h, (4c, kw)] -> ubuf [h, c, kwstack]
                     (ui bins 0,64 dropped)
  T3  DMA-xbar transpose (split): ubuf -> s5rhs [kwstack, c, h]
  S5  irfft over W:  lhsT=Ainv [kstack, w], rhs=s5rhs + t1 (residual in
                     spectrum: irfft_W(rfft_W(x)) == x), 4c-chunks
                     -> psum [w, (4c, 128h)] -> out [w, c, h] f32

Host passes x pre-transposed to [B, W, 96, H] and un-transposes the
[B, W, 96, H] output, so all DMAs are fully contiguous per partition.
"""
import numpy as np
import ml_dtypes

B, H, W, C = 4, 128, 128, 768
NB, BL = 8, 96
WF = 65
LAMBD = 0.01
BF16 = ml_dtypes.bfloat16

_CACHE = {}


def _make_consts():
    w = np.arange(W, dtype=np.float64)[:, None]
    k = np.arange(WF, dtype=np.float64)[None, :]
    th = 2 * np.pi * w * k / W
    s = 1.0 / np.sqrt(W)
    f1 = np.concatenate([np.cos(th) * s, -np.sin(th[:, 1:64]) * s], axis=1)

    h = np.arange(H, dtype=np.float64)[:, None]
    kh = np.arange(H, dtype=np.float64)[None, :]
    th2 = 2 * np.pi * h * kh / H
    c2 = np.cos(th2) / np.sqrt(H)
    s2 = np.sin(th2) / np.sqrt(H)
    f2a = np.concatenate([c2, -s2], axis=1)   # rhs when lhsT = t_r
    f2b = np.concatenate([s2, c2], axis=1)    # rhs when lhsT = t_i

    gc = (np.cos(th2) / np.sqrt(H)).T         # [kh, h]
    gs = (np.sin(th2) / np.sqrt(H)).T

    kk = np.arange(WF, dtype=np.float64)[:, None]
    ww = np.arange(W, dtype=np.float64)[None, :]
    th3 = 2 * np.pi * kk * ww / W
    beta = np.full((WF, 1), 2.0); beta[0] = 1.0; beta[64] = 1.0
    ac = beta * np.cos(th3) / np.sqrt(W)
    asn = -2.0 * np.sin(th3[1:64]) / np.sqrt(W)
    ainv = np.concatenate([ac, asn], axis=0)

    cast = lambda a: np.ascontiguousarray(a).astype(BF16)
    return dict(f1=cast(f1), f2a=cast(f2a), f2b=cast(f2b),
                gc=cast(gc), gs=cast(gs), gsn=cast(-gs), ainv=cast(ainv))


def _groups():
    gs = [list(range(i, i + 8)) for i in range(0, 64, 8)]
    gs.append([64])
    return gs


def _build():
    from contextlib import ExitStack
    from concourse import bacc, mybir, tile

    dt = mybir.dt
    nc = bacc.Bacc("TRN2", target_bir_lowering=False, debug=False, num_devices=8)

    x_d = nc.dram_tensor("x", [B, W, BL, H], dt.bfloat16, kind="ExternalInput")
    f1_d = nc.dram_tensor("f1", [128, 128], dt.bfloat16, kind="ExternalInput")
    f2a_d = nc.dram_tensor("f2a", [128, 256], dt.bfloat16, kind="ExternalInput")
    f2b_d = nc.dram_tensor("f2b", [128, 256], dt.bfloat16, kind="ExternalInput")
    gc_d = nc.dram_tensor("gc", [128, 128], dt.bfloat16, kind="ExternalInput")
    gs_d = nc.dram_tensor("gs", [128, 128], dt.bfloat16, kind="ExternalInput")
    gsn_d = nc.dram_tensor("gsn", [128, 128], dt.bfloat16, kind="ExternalInput")
    ainv_d = nc.dram_tensor("ainv", [128, 128], dt.bfloat16, kind="ExternalInput")
    w1r_d = nc.dram_tensor("w1r", [BL, 97], dt.bfloat16, kind="ExternalInput")
    w1i_d = nc.dram_tensor("w1i", [BL, 97], dt.bfloat16, kind="ExternalInput")
    w1in_d = nc.dram_tensor("w1in", [BL, 97], dt.bfloat16, kind="ExternalInput")
    w2a_d = nc.dram_tensor("w2a", [97, 192], dt.bfloat16, kind="ExternalInput")
    w2b_d = nc.dram_tensor("w2b", [97, 192], dt.bfloat16, kind="ExternalInput")
    b1r_d = nc.dram_tensor("b1r", [97, 1], dt.float32, kind="ExternalInput")
    b1i_d = nc.dram_tensor("b1i", [97, 1], dt.float32, kind="ExternalInput")
    out_d = nc.dram_tensor("out", [B, W, BL, H], dt.float32, kind="ExternalOutput")

    Relu = mybir.ActivationFunctionType.Relu
    Ident = mybir.ActivationFunctionType.Identity
    ADD = mybir.AluOpType.add
    MAX = mybir.AluOpType.max
    MIN = mybir.AluOpType.min

    with tile.TileContext(nc) as tc, ExitStack() as ctx:
        cp = ctx.enter_context(tc.tile_pool(name="const", bufs=1))
        xp = ctx.enter_context(tc.tile_pool(name="xb", bufs=1))
        t1p = ctx.enter_context(tc.tile_pool(name="t1", bufs=2))
        t2p = ctx.enter_context(tc.tile_pool(name="t2", bufs=1))
        sw = ctx.enter_context(tc.tile_pool(name="specw", bufs=2))
        o1p = ctx.enter_context(tc.tile_pool(name="o1w", bufs=2))
        o2p = ctx.enter_context(tc.tile_pool(name="o2w", bufs=1))
        tap = ctx.enter_context(tc.tile_pool(name="tmpa", bufs=2))
        tbp = ctx.enter_context(tc.tile_pool(name="tmpb", bufs=2))
        up = ctx.enter_context(tc.tile_pool(name="ubuf", bufs=1))
        s5p = ctx.enter_context(tc.tile_pool(name="s5rhs", bufs=1))
        ocp = ctx.enter_context(tc.tile_pool(name="outc", bufs=2))
        psm = ctx.enter_context(tc.tile_pool(name="psmain", bufs=4, space="PSUM"))
        ps2p = ctx.enter_context(tc.tile_pool(name="ps2", bufs=2, space="PSUM"))
        pm2p = ctx.enter_context(tc.tile_pool(name="psm2", bufs=2, space="PSUM"))

        def cload(dram, shape, dtype=dt.bfloat16):
            t = cp.tile(shape, dtype, tag=f"c_{dram.name}")
            nc.sync.dma_start(t[:], dram[:])
            return t

        f1 = cload(f1_d, [128, 128]); f2a = cload(f2a_d, [128, 256])
        f2b = cload(f2b_d, [128, 256]); gc = cload(gc_d, [128, 128])
        gs = cload(gs_d, [128, 128]); gsn = cload(gsn_d, [128, 128])
        ainv = cload(ainv_d, [128, 128])
        w1r = cload(w1r_d, [BL, 97]); w1i = cload(w1i_d, [BL, 97])
        w1in = cload(w1in_d, [BL, 97])
        w2a = cload(w2a_d, [97, 192]); w2b = cload(w2b_d, [97, 192])
        b1r = cload(b1r_d, [97, 1], dt.float32)
        b1i = cload(b1i_d, [97, 1], dt.float32)
        lamneg = cp.tile([128, 1], dt.float32, tag="c_lamneg")
        nc.gpsimd.memset(lamneg[:], -LAMBD)
        zbias = cp.tile([128, 1], dt.float32, tag="c_zbias")
        nc.gpsimd.memset(zbias[:], 0.0)

        GROUPS = _groups()

        _flip = [0]

        def split_evict(dst_dve, src_dve, dst_act, src_act, act_bias=None,
                        act_func=None):
            # whole-tile eviction, alternating engines 3:2 DVE:ACT (two
            # engines reading halves of one PSUM tile raced on hardware)
            _flip[0] ^= 1
            if _flip[0]:
                nc.vector.tensor_copy(dst_dve, src_dve)
                nc.vector.tensor_copy(dst_act, src_act)
            else:
                p = src_dve.shape[0]
                nc.scalar.activation(dst_dve, src_dve, Ident, bias=zbias[0:p, :])
                nc.scalar.activation(dst_act, src_act, Ident, bias=zbias[0:p, :])

        prev = None  # (t1, s5r) of previous batch, for interleaved S5

        def emit_s5_group(st, gi):
            # 3 S5 chunks (12 c-columns) + 1 out-DMA per kw-group slot
            if st is None or gi >= 8:
                return
            t1q, s5q, bq = st
            co = gi * 12
            oc = ocp.tile([128, 12, 128], dt.float32, tag="oc")  # [w, c, h]
            for ci in range(co, co + 12, 4):
                ps5 = psm.tile([128, 4, 128], dt.float32, tag="ps")
                nc.tensor.matmul(ps5[:], ainv[:], s5q[:, ci:ci + 4, :],
                                 start=True, stop=False)
                nc.tensor.matmul(ps5[:], ainv[:], t1q[:, ci:ci + 4, :],
                                 start=False, stop=True)
                split_evict(oc[:, ci - co:ci - co + 2, :], ps5[:, 0:2, :],
                            oc[:, ci - co + 2:ci - co + 4, :], ps5[:, 2:4, :])
            nc.sync.dma_start(out_d[bq, :, co:co + 12, :], oc[:])

        for b in range(B + 1):
            if b == B:
                for gi in range(8):
                    emit_s5_group(prev, gi)
                break
            xb = xp.tile([128, BL, 128], dt.bfloat16, tag="xb")  # [w, c, h]
            nc.sync.dma_start(xb[:], x_d[b])

            # ---- S1: 4c chunks, contiguous rhs + contiguous eviction
            t1 = t1p.tile([128, BL, 128], dt.bfloat16, tag="t1")  # [kw, c, h]
            for ci in range(0, BL, 4):
                ps = psm.tile([128, 4, 128], dt.float32, tag="ps")
                nc.tensor.matmul(ps[:], f1[:], xb[:, ci:ci + 4, :],
                                 start=True, stop=True)
                split_evict(t1[:, ci:ci + 2, :], ps[:, 0:2, :],
                            t1[:, ci + 2:ci + 4, :], ps[:, 2:4, :])

            # ---- T1 (halves on the two hwdge engines)
            t2 = t2p.tile([128, BL, 128], dt.bfloat16, tag="t2")  # [h, c, kwstack]
            nc.sync.dma_start_transpose(t2[:, 0:48, :], t1[:, 0:48, :])
            nc.scalar.dma_start_transpose(t2[:, 48:BL, :], t1[:, 48:BL, :])

            # ---- middle section per kw-group, interleaved with S5(b-1)
            ub = up.tile([128, BL, 128], dt.bfloat16, tag="ub")  # [h, c, kwstack]
            o2 = o2p.tile([128, WF, 2, BL], dt.bfloat16, tag="o2")  # [kh,kw,ri,c]
            for gi, grp in enumerate(GROUPS):
                g0, gl = grp[0], len(grp)
                spec = sw.tile([BL, 8, 2, 128], dt.bfloat16, tag="spec")
                for j0 in range(0, gl, 2):
                    jl = min(2, gl - j0)
                    ps2 = ps2p.tile([BL, 2, 2, 128], dt.float32, tag="ps2")
                    for j in range(j0, j0 + jl):
                        kw = g0 + j
                        edge = kw in (0, 64)
                        nc.tensor.matmul(ps2[:, j - j0, :, :], t2[:, :, kw],
                                         f2a[:], start=True, stop=edge)
                        if not edge:
                            nc.tensor.matmul(ps2[:, j - j0, :, :],
                                             t2[:, :, 64 + kw], f2b[:],
                                             start=False, stop=True)
                    split_evict(spec[:, j0:j0 + jl, 0, :], ps2[:, 0:jl, 0, :],
                                spec[:, j0:j0 + jl, 1, :], ps2[:, 0:jl, 1, :])
                # MLP1 (M=97: row 96 == 1.0 bias row)
                o1 = o1p.tile([97, 2, 8, 128], dt.bfloat16, tag="o1")
                for c0 in range(0, gl, 4):
                    cl = min(4, gl - c0)
                    xr = spec[:, c0:c0 + cl, 0, :]
                    xi = spec[:, c0:c0 + cl, 1, :]
                    pr = psm.tile([97, 4, 128], dt.float32, tag="ps")
                    nc.tensor.matmul(pr[:, 0:cl, :], w1r[:], xr, start=True, stop=False)
                    nc.tensor.matmul(pr[:, 0:cl, :], w1in[:], xi, start=False, stop=True)
                    pi = psm.tile([97, 4, 128], dt.float32, tag="ps")
                    nc.tensor.matmul(pi[:, 0:cl, :], w1i[:], xr, start=True, stop=False)
                    nc.tensor.matmul(pi[:, 0:cl, :], w1r[:], xi, start=False, stop=True)
                    nc.scalar.activation(o1[:, 0, c0:c0 + cl, :], pr[:, 0:cl, :],
                                         Relu, bias=b1r[:])
                    nc.scalar.activation(o1[:, 1, c0:c0 + cl, :], pi[:, 0:cl, :],
                                         Relu, bias=b1i[:])
                # MLP2 (bias via o1 ones-row) + softshrink -> o2 [kh, ri, c, kw]
                for j0 in range(0, gl, 2):
                    jl = min(2, gl - j0)
                    pm = pm2p.tile([128, 2, 2, BL], dt.float32, tag="pm2")
                    for j in range(j0, j0 + jl):
                        nc.tensor.matmul(pm[:, j - j0, :, :], o1[:, 0, j, :],
                                         w2a[:], start=True, stop=False)
                        nc.tensor.matmul(pm[:, j - j0, :, :], o1[:, 1, j, :],
                                         w2b[:], start=False, stop=True)
                    ta = tap.tile([128, 2, 2, BL], dt.bfloat16, tag="ta")
                    tb = tbp.tile([128, 2, 2, BL], dt.bfloat16, tag="tb")
                    nc.scalar.activation(ta[:, 0:jl], pm[:, 0:jl], Relu,
                                         bias=lamneg[:])
                    nc.vector.tensor_scalar(tb[:, 0:jl], pm[:, 0:jl],
                                            LAMBD, 0.0, ADD, MIN)
                    nc.gpsimd.tensor_tensor(o2[:, g0 + j0:g0 + j0 + jl, :, :],
                                            ta[:, 0:jl], tb[:, 0:jl], ADD)
                # interleave previous batch's S5 work
                emit_s5_group(prev, gi)

            # ---- S4: 4c chunks; rhs 65-el runs; evictions 65/63-el runs
            for ci in range(0, BL, 4):
                o2r = o2[:, :, 0, ci:ci + 4].rearrange("k w c -> k c w")
                o2i = o2[:, :, 1, ci:ci + 4].rearrange("k w c -> k c w")
                pu = psm.tile([128, 4, WF], dt.float32, tag="ps")
                nc.tensor.matmul(pu[:], gc[:], o2r, start=True, stop=False)
                nc.tensor.matmul(pu[:], gsn[:], o2i, start=False, stop=True)
                split_evict(ub[:, ci:ci + 2, 0:WF], pu[:, 0:2, :],
                            ub[:, ci + 2:ci + 4, 0:WF], pu[:, 2:4, :])
                pv = psm.tile([128, 4, 64], dt.float32, tag="ps")
                nc.tensor.matmul(pv[:], gs[:],
                                 o2[:, 0:64, 0, ci:ci + 4].rearrange("k w c -> k c w"),
                                 start=True, stop=False)
                nc.tensor.matmul(pv[:], gc[:],
                                 o2[:, 0:64, 1, ci:ci + 4].rearrange("k w c -> k c w"),
                                 start=False, stop=True)
                split_evict(ub[:, ci:ci + 2, WF:128], pv[:, 0:2, 1:64],
                            ub[:, ci + 2:ci + 4, WF:128], pv[:, 2:4, 1:64])

            # ---- T3 (halves)
            s5r = s5p.tile([128, BL, 128], dt.bfloat16, tag="s5r")  # [kstack, c, h]
            nc.sync.dma_start_transpose(s5r[:, 0:48, :], ub[:, 0:48, :])
            nc.scalar.dma_start_transpose(s5r[:, 48:BL, :], ub[:, 48:BL, :])
            prev = (t1, s5r, b)

    nc.compile()
    return nc


def get_nc():
    if "nc" not in _CACHE:
        _CACHE["nc"] = _build()
    return _CACHE["nc"]


def make_in_maps(x, w1, b1, w2, b2):
    consts = _make_consts()
    f32 = np.float32
    in_maps = []
    for i in range(NB):
        sl = slice(BL * i, BL * (i + 1))
        m = dict(consts)
        m["x"] = np.ascontiguousarray(
            x[..., sl].transpose(0, 2, 3, 1)).astype(BF16)  # [B, W, 96, H]
        w1p = np.zeros((2, BL, 97), np.float64)
        w1p[:, :, 0:BL] = w1[:, i]
        m["w1r"] = w1p[0].astype(BF16)
        m["w1i"] = w1p[1].astype(BF16)
        m["w1in"] = (-w1p[1]).astype(BF16)
        w2a = np.zeros((97, 192), np.float64)
        w2a[0:BL, 0:BL] = w2[0, i]; w2a[0:BL, BL:] = w2[1, i]
        w2a[BL, 0:BL] = b2[0, i]; w2a[BL, BL:] = b2[1, i]
        w2bp = np.zeros((97, 192), np.float64)
        w2bp[0:BL, 0:BL] = -w2[1, i]; w2bp[0:BL, BL:] = w2[0, i]
        m["w2a"] = w2a.astype(BF16)
        m["w2b"] = w2bp.astype(BF16)
        b1p = np.zeros((2, 97, 1), np.float64)
        b1p[:, 0:BL, 0] = b1[:, i]
        b1p[0, BL, 0] = 1.0  # ones row for fused MLP2 bias
        m["b1r"] = b1p[0].astype(f32)
        m["b1i"] = b1p[1].astype(f32)
        in_maps.append(m)
    return in_maps


def kernel(x, w1, b1, w2, b2):
    from concourse.bass_utils import run_bass_kernel_spmd
    nc = get_nc()
    in_maps = make_in_maps(np.asarray(x), np.asarray(w1), np.asarray(b1),
                           np.asarray(w2), np.asarray(b2))
    res = run_bass_kernel_spmd(nc, in_maps, core_ids=list(range(NB)))
    out = np.concatenate(
        [res.results[i]["out"].transpose(0, 3, 1, 2) for i in range(NB)],
        axis=-1)
    return out.astype(np.float32)


# revision 23
# speedup vs baseline: 1.0711x; 1.0175x over previous
"""AFNO2D Trainium2 kernel (8 NeuronCores, channel-sharded, zero collectives).

Each core processes one 96-channel block (FFT is per-channel; the MLP is
block-diagonal with exactly 8 blocks of 96 -> core i owns block i end-to-end).

Per-core pipeline (all matmuls bf16, fp32 PSUM). Layouts are chosen so that
every matmul rhs is contiguous-innermost and every PSUM eviction writes
contiguous (or long-run) destinations; evictions are split across DVE+ACT.

  S1  rfft over W:   lhsT=F1 [w,128]=[cos(65)|-sin(63)], rhs=xb [w, c, h]
                     4c-chunks -> psum [kwstack,(4c,128h)] -> t1 [kw, c, h]
  T1  DMA-xbar transpose (split on 2 engines): t1 -> t2 [h, c, kwstack]
  S2  DFT over H (data stationary): per kw: lhsT=t2[:, :, kw*] [h, c],
                     rhs=F2a/F2b [h, 256] -> psum [c, (khr|khi)] -> specw
  MLP1 (rhs mode):   lhsT=w1* [96,97] (col 96 zero, b1 row 96 = 1 ->
                     o1 row 96 == 1, the fused bias row for MLP2),
                     relu+b1 evict -> o1w [97, ri, kw, kh]
  MLP2 (data stationary): per kw: lhsT=o1w slices [97, kh],
                     rhs=[w2r|w2i ; b2r|b2i] / [-w2i|w2r ; 0]
                     -> psum [kh, (kw, cr|ci)]
  softshrink:        relu(v-l) + min(v+l, 0) -> o2 [kh, ri, c, kw]
  S4  iDFT over H (rhs mode): lhsT=Gc/Gs/-Gs, rhs=o2 4c-chunks (65-el runs)
                     -> psum [h, (4c, kw)] -> ubuf [h, c, kwstack]
                     (ui bins 0,64 dropped)
  T3  DMA-xbar transpose (split): ubuf -> s5rhs [kwstack, c, h]
  S5  irfft over W:  lhsT=Ainv [kstack, w], rhs=s5rhs + t1 (residual in
                     spectrum: irfft_W(rfft_W(x)) == x), 4c-chunks
                     -> psum [w, (4c, 128h)] -> out [w, c, h] f32

Host passes x pre-transposed to [B, W, 96, H] and un-transposes the
[B, W, 96, H] output, so all DMAs are fully contiguous per partition.
"""
import numpy as np
import ml_dtypes

B, H, W, C = 4, 128, 128, 768
NB, BL = 8, 96
WF = 65
LAMBD = 0.01
BF16 = ml_dtypes.bfloat16

_CACHE = {}


def _make_consts():
    w = np.arange(W, dtype=np.float64)[:, None]
    k = np.arange(WF, dtype=np.float64)[None, :]
    th = 2 * np.pi * w * k / W
    s = 1.0 / np.sqrt(W)
    f1 = np.concatenate([np.cos(th) * s, -np.sin(th[:, 1:64]) * s], axis=1)

    h = np.arange(H, dtype=np.float64)[:, None]
    kh = np.arange(H, dtype=np.float64)[None, :]
    th2 = 2 * np.pi * h * kh / H
    c2 = np.cos(th2) / np.sqrt(H)
    s2 = np.sin(th2) / np.sqrt(H)
    f2a = np.concatenate([c2, -s2], axis=1)   # rhs when lhsT = t_r
    f2b = np.concatenate([s2, c2], axis=1)    # rhs when lhsT = t_i

    gc = (np.cos(th2) / np.sqrt(H)).T         # [kh, h]
    gs = (np.sin(th2) / np.sqrt(H)).T

    kk = np.arange(WF, dtype=np.float64)[:, None]
    ww = np.arange(W, dtype=np.float64)[None, :]
    th3 = 2 * np.pi * kk * ww / W
    beta = np.full((WF, 1), 2.0); beta[0] = 1.0; beta[64] = 1.0
    ac = beta * np.cos(th3) / np.sqrt(W)
    asn = -2.0 * np.sin(th3[1:64]) / np.sqrt(W)
    ainv = np.concatenate([ac, asn], axis=0)

    cast = lambda a: np.ascontiguousarray(a).astype(BF16)
    return dict(f1=cast(f1), f2a=cast(f2a), f2b=cast(f2b),
                gc=cast(gc), gs=cast(gs), gsn=cast(-gs), ainv=cast(ainv))


def _groups():
    gs = [list(range(i, i + 8)) for i in range(0, 64, 8)]
    gs.append([64])
    return gs


def _build():
    from contextlib import ExitStack
    from concourse import bacc, mybir, tile

    dt = mybir.dt
    nc = bacc.Bacc("TRN2", target_bir_lowering=False, debug=False, num_devices=8)

    x_d = nc.dram_tensor("x", [B, W, BL, H], dt.bfloat16, kind="ExternalInput")
    f1_d = nc.dram_tensor("f1", [128, 128], dt.bfloat16, kind="ExternalInput")
    f2a_d = nc.dram_tensor("f2a", [128, 256], dt.bfloat16, kind="ExternalInput")
    f2b_d = nc.dram_tensor("f2b", [128, 256], dt.bfloat16, kind="ExternalInput")
    gc_d = nc.dram_tensor("gc", [128, 128], dt.bfloat16, kind="ExternalInput")
    gs_d = nc.dram_tensor("gs", [128, 128], dt.bfloat16, kind="ExternalInput")
    gsn_d = nc.dram_tensor("gsn", [128, 128], dt.bfloat16, kind="ExternalInput")
    ainv_d = nc.dram_tensor("ainv", [128, 128], dt.bfloat16, kind="ExternalInput")
    w1r_d = nc.dram_tensor("w1r", [BL, 97], dt.bfloat16, kind="ExternalInput")
    w1i_d = nc.dram_tensor("w1i", [BL, 97], dt.bfloat16, kind="ExternalInput")
    w1in_d = nc.dram_tensor("w1in", [BL, 97], dt.bfloat16, kind="ExternalInput")
    w2a_d = nc.dram_tensor("w2a", [97, 192], dt.bfloat16, kind="ExternalInput")
    w2b_d = nc.dram_tensor("w2b", [97, 192], dt.bfloat16, kind="ExternalInput")
    b1r_d = nc.dram_tensor("b1r", [97, 1], dt.float32, kind="ExternalInput")
    b1i_d = nc.dram_tensor("b1i", [97, 1], dt.float32, kind="ExternalInput")
    out_d = nc.dram_tensor("out", [B, W, BL, H], dt.float32, kind="ExternalOutput")

    Relu = mybir.ActivationFunctionType.Relu
    Ident = mybir.ActivationFunctionType.Identity
    ADD = mybir.AluOpType.add
    MAX = mybir.AluOpType.max
    MIN = mybir.AluOpType.min

    with tile.TileContext(nc) as tc, ExitStack() as ctx:
        cp = ctx.enter_context(tc.tile_pool(name="const", bufs=1))
        xp = ctx.enter_context(tc.tile_pool(name="xb", bufs=1))
        t1p = ctx.enter_context(tc.tile_pool(name="t1", bufs=2))
        t2p = ctx.enter_context(tc.tile_pool(name="t2", bufs=1))
        sw = ctx.enter_context(tc.tile_pool(name="specw", bufs=2))
        o1p = ctx.enter_context(tc.tile_pool(name="o1w", bufs=2))
        o2p = ctx.enter_context(tc.tile_pool(name="o2w", bufs=1))
        tap = ctx.enter_context(tc.tile_pool(name="tmpa", bufs=2))
        tbp = ctx.enter_context(tc.tile_pool(name="tmpb", bufs=2))
        up = ctx.enter_context(tc.tile_pool(name="ubuf", bufs=1))
        s5p = ctx.enter_context(tc.tile_pool(name="s5rhs", bufs=1))
        ocp = ctx.enter_context(tc.tile_pool(name="outc", bufs=2))
        psm = ctx.enter_context(tc.tile_pool(name="psmain", bufs=4, space="PSUM"))
        ps2p = ctx.enter_context(tc.tile_pool(name="ps2", bufs=2, space="PSUM"))
        pm2p = ctx.enter_context(tc.tile_pool(name="psm2", bufs=2, space="PSUM"))

        def cload(dram, shape, dtype=dt.bfloat16):
            t = cp.tile(shape, dtype, tag=f"c_{dram.name}")
            nc.sync.dma_start(t[:], dram[:])
            return t

        f1 = cload(f1_d, [128, 128]); f2a = cload(f2a_d, [128, 256])
        f2b = cload(f2b_d, [128, 256]); gc = cload(gc_d, [128, 128])
        gs = cload(gs_d, [128, 128]); gsn = cload(gsn_d, [128, 128])
        ainv = cload(ainv_d, [128, 128])
        w1r = cload(w1r_d, [BL, 97]); w1i = cload(w1i_d, [BL, 97])
        w1in = cload(w1in_d, [BL, 97])
        w2a = cload(w2a_d, [97, 192]); w2b = cload(w2b_d, [97, 192])
        b1r = cload(b1r_d, [97, 1], dt.float32)
        b1i = cload(b1i_d, [97, 1], dt.float32)
        lamneg = cp.tile([128, 1], dt.float32, tag="c_lamneg")
        nc.gpsimd.memset(lamneg[:], -LAMBD)
        zbias = cp.tile([128, 1], dt.float32, tag="c_zbias")
        nc.gpsimd.memset(zbias[:], 0.0)

        GROUPS = _groups()

        _flip = [0]

        def split_evict(dst_dve, src_dve, dst_act, src_act, act_bias=None,
                        act_func=None):
            # whole-tile eviction, alternating engines 3:2 DVE:ACT (two
            # engines reading halves of one PSUM tile raced on hardware)
            _flip[0] ^= 1
            if _flip[0]:
                nc.vector.tensor_copy(dst_dve, src_dve)
                nc.vector.tensor_copy(dst_act, src_act)
            else:
                p = src_dve.shape[0]
                nc.scalar.activation(dst_dve, src_dve, Ident, bias=zbias[0:p, :])
                nc.scalar.activation(dst_act, src_act, Ident, bias=zbias[0:p, :])

        prev = None  # (t1, s5r) of previous batch, for interleaved S5

        def emit_s5_group(st, gi):
            # 3 S5 chunks (12 c-columns) + 1 out-DMA per kw-group slot
            if st is None or gi >= 8:
                return
            t1q, s5q, bq = st
            co = gi * 12
            oc = ocp.tile([128, 12, 128], dt.float32, tag="oc")  # [w, c, h]
            for ci in range(co, co + 12, 4):
                ps5 = psm.tile([128, 4, 128], dt.float32, tag="ps")
                nc.tensor.matmul(ps5[:], ainv[:], s5q[:, ci:ci + 4, :],
                                 start=True, stop=False)
                nc.tensor.matmul(ps5[:], ainv[:], t1q[:, ci:ci + 4, :],
                                 start=False, stop=True)
                split_evict(oc[:, ci - co:ci - co + 2, :], ps5[:, 0:2, :],
                            oc[:, ci - co + 2:ci - co + 4, :], ps5[:, 2:4, :])
            nc.sync.dma_start(out_d[bq, :, co:co + 12, :], oc[:])

        for b in range(B + 1):
            if b == B:
                for gi in range(8):
                    emit_s5_group(prev, gi)
                break
            xb = xp.tile([128, BL, 128], dt.bfloat16, tag="xb")  # [w, c, h]
            nc.sync.dma_start(xb[:], x_d[b])

            # ---- S1: 4c chunks, contiguous rhs + contiguous eviction
            t1 = t1p.tile([128, BL, 128], dt.bfloat16, tag="t1")  # [kw, c, h]
            for ci in range(0, BL, 4):
                ps = psm.tile([128, 4, 128], dt.float32, tag="ps")
                nc.tensor.matmul(ps[:], f1[:], xb[:, ci:ci + 4, :],
                                 start=True, stop=True)
                split_evict(t1[:, ci:ci + 2, :], ps[:, 0:2, :],
                            t1[:, ci + 2:ci + 4, :], ps[:, 2:4, :])

            # ---- T1 (halves on the two hwdge engines)
            t2 = t2p.tile([128, BL, 128], dt.bfloat16, tag="t2")  # [h, c, kwstack]
            nc.sync.dma_start_transpose(t2[:, 0:48, :], t1[:, 0:48, :])
            nc.scalar.dma_start_transpose(t2[:, 48:BL, :], t1[:, 48:BL, :])

            # ---- middle section per kw-group, interleaved with S5(b-1)
            o2 = o2p.tile([128, 2, BL, WF], dt.bfloat16, tag="o2")  # [kh,ri,c,kw]
            scr = up.tile([128, WF, 2, BL], dt.bfloat16, tag="ub")  # [kh,kw,ri,c]
            for gi, grp in enumerate(GROUPS):
                g0, gl = grp[0], len(grp)
                spec = sw.tile([BL, 8, 2, 128], dt.bfloat16, tag="spec")
                for j0 in range(0, gl, 2):
                    jl = min(2, gl - j0)
                    ps2 = ps2p.tile([BL, 2, 2, 128], dt.float32, tag="ps2")
                    for j in range(j0, j0 + jl):
                        kw = g0 + j
                        edge = kw in (0, 64)
                        nc.tensor.matmul(ps2[:, j - j0, :, :], t2[:, :, kw],
                                         f2a[:], start=True, stop=edge)
                        if not edge:
                            nc.tensor.matmul(ps2[:, j - j0, :, :],
                                             t2[:, :, 64 + kw], f2b[:],
                                             start=False, stop=True)
                    split_evict(spec[:, j0:j0 + jl, 0, :], ps2[:, 0:jl, 0, :],
                                spec[:, j0:j0 + jl, 1, :], ps2[:, 0:jl, 1, :])
                # MLP1 (M=97: row 96 == 1.0 bias row)
                o1 = o1p.tile([97, 2, 8, 128], dt.bfloat16, tag="o1")
                for c0 in range(0, gl, 4):
                    cl = min(4, gl - c0)
                    xr = spec[:, c0:c0 + cl, 0, :]
                    xi = spec[:, c0:c0 + cl, 1, :]
                    pr = psm.tile([97, 4, 128], dt.float32, tag="ps")
                    nc.tensor.matmul(pr[:, 0:cl, :], w1r[:], xr, start=True, stop=False)
                    nc.tensor.matmul(pr[:, 0:cl, :], w1in[:], xi, start=False, stop=True)
                    pi = psm.tile([97, 4, 128], dt.float32, tag="ps")
                    nc.tensor.matmul(pi[:, 0:cl, :], w1i[:], xr, start=True, stop=False)
                    nc.tensor.matmul(pi[:, 0:cl, :], w1r[:], xi, start=False, stop=True)
                    nc.scalar.activation(o1[:, 0, c0:c0 + cl, :], pr[:, 0:cl, :],
                                         Relu, bias=b1r[:])
                    nc.vector.tensor_scalar(o1[:, 1, c0:c0 + cl, :],
                                            pi[:, 0:cl, :], b1i[:], 0.0,
                                            ADD, MAX)
                # MLP2 (bias via o1 ones-row) + softshrink -> o2 [kh, ri, c, kw]
                for j0 in range(0, gl, 2):
                    jl = min(2, gl - j0)
                    pm = pm2p.tile([128, 2, 2, BL], dt.float32, tag="pm2")
                    for j in range(j0, j0 + jl):
                        nc.tensor.matmul(pm[:, j - j0, :, :], o1[:, 0, j, :],
                                         w2a[:], start=True, stop=False)
                        nc.tensor.matmul(pm[:, j - j0, :, :], o1[:, 1, j, :],
                                         w2b[:], start=False, stop=True)
                    ta = tap.tile([128, 2, 2, BL], dt.bfloat16, tag="ta")
                    tb = tbp.tile([128, 2, 2, BL], dt.bfloat16, tag="tb")
                    nc.scalar.activation(ta[:, 0:jl], pm[:, 0:jl], Relu,
                                         bias=lamneg[:])
                    nc.vector.tensor_scalar(tb[:, 0:jl], pm[:, 0:jl],
                                            LAMBD, 0.0, ADD, MIN)
                    nc.gpsimd.tensor_tensor(scr[:, g0 + j0:g0 + j0 + jl, :, :],
                                            ta[:, 0:jl], tb[:, 0:jl], ADD)
                # interleave previous batch's S5 work
                emit_s5_group(prev, gi)

            # ---- reorder scratch [kh,kw,ri,c] -> o2 [kh,ri,c,kw]
            for ri in range(2):
                for qi, q in enumerate(range(0, BL, 24)):
                    src = scr[:, :, ri, q:q + 24].rearrange("k w c -> k c w")
                    eng = (nc.gpsimd, nc.gpsimd, nc.vector, nc.scalar)[qi]
                    if eng is nc.scalar:
                        nc.scalar.activation(o2[:, ri, q:q + 24, :], src,
                                             Ident, bias=zbias[:])
                    else:
                        eng.tensor_copy(o2[:, ri, q:q + 24, :], src)
            # ---- S4: 4c chunks; rhs 65-el runs; evictions 65/63-el runs
            ub = up.tile([128, BL, 128], dt.bfloat16, tag="ub")  # [h, c, kwstack]
            for ci in range(0, BL, 4):
                o2r = o2[:, 0, ci:ci + 4, :]
                o2i = o2[:, 1, ci:ci + 4, :]
                pu = psm.tile([128, 4, WF], dt.float32, tag="ps")
                nc.tensor.matmul(pu[:], gc[:], o2r, start=True, stop=False)
                nc.tensor.matmul(pu[:], gsn[:], o2i, start=False, stop=True)
                split_evict(ub[:, ci:ci + 2, 0:WF], pu[:, 0:2, :],
                            ub[:, ci + 2:ci + 4, 0:WF], pu[:, 2:4, :])
                pv = psm.tile([128, 4, 64], dt.float32, tag="ps")
                nc.tensor.matmul(pv[:], gs[:], o2r[:, :, 0:64],
                                 start=True, stop=False)
                nc.tensor.matmul(pv[:], gc[:], o2i[:, :, 0:64],
                                 start=False, stop=True)
                split_evict(ub[:, ci:ci + 2, WF:128], pv[:, 0:2, 1:64],
                            ub[:, ci + 2:ci + 4, WF:128], pv[:, 2:4, 1:64])

            # ---- T3 (halves)
            s5r = s5p.tile([128, BL, 128], dt.bfloat16, tag="s5r")  # [kstack, c, h]
            nc.sync.dma_start_transpose(s5r[:, 0:48, :], ub[:, 0:48, :])
            nc.scalar.dma_start_transpose(s5r[:, 48:BL, :], ub[:, 48:BL, :])
            prev = (t1, s5r, b)

    nc.compile()
    return nc


def get_nc():
    if "nc" not in _CACHE:
        _CACHE["nc"] = _build()
    return _CACHE["nc"]


def make_in_maps(x, w1, b1, w2, b2):
    consts = _make_consts()
    f32 = np.float32
    in_maps = []
    for i in range(NB):
        sl = slice(BL * i, BL * (i + 1))
        m = dict(consts)
        m["x"] = np.ascontiguousarray(
            x[..., sl].transpose(0, 2, 3, 1)).astype(BF16)  # [B, W, 96, H]
        w1p = np.zeros((2, BL, 97), np.float64)
        w1p[:, :, 0:BL] = w1[:, i]
        m["w1r"] = w1p[0].astype(BF16)
        m["w1i"] = w1p[1].astype(BF16)
        m["w1in"] = (-w1p[1]).astype(BF16)
        w2a = np.zeros((97, 192), np.float64)
        w2a[0:BL, 0:BL] = w2[0, i]; w2a[0:BL, BL:] = w2[1, i]
        w2a[BL, 0:BL] = b2[0, i]; w2a[BL, BL:] = b2[1, i]
        w2bp = np.zeros((97, 192), np.float64)
        w2bp[0:BL, 0:BL] = -w2[1, i]; w2bp[0:BL, BL:] = w2[0, i]
        m["w2a"] = w2a.astype(BF16)
        m["w2b"] = w2bp.astype(BF16)
        b1p = np.zeros((2, 97, 1), np.float64)
        b1p[:, 0:BL, 0] = b1[:, i]
        b1p[0, BL, 0] = 1.0  # ones row for fused MLP2 bias
        m["b1r"] = b1p[0].astype(f32)
        m["b1i"] = b1p[1].astype(f32)
        in_maps.append(m)
    return in_maps


def kernel(x, w1, b1, w2, b2):
    from concourse.bass_utils import run_bass_kernel_spmd
    nc = get_nc()
    in_maps = make_in_maps(np.asarray(x), np.asarray(w1), np.asarray(b1),
                           np.asarray(w2), np.asarray(b2))
    res = run_bass_kernel_spmd(nc, in_maps, core_ids=list(range(NB)))
    out = np.concatenate(
        [res.results[i]["out"].transpose(0, 3, 1, 2) for i in range(NB)],
        axis=-1)
    return out.astype(np.float32)


# revision 24
# speedup vs baseline: 1.1906x; 1.1116x over previous
"""AFNO2D Trainium2 kernel (8 NeuronCores, channel-sharded, zero collectives).

Each core processes one 96-channel block (FFT is per-channel; the MLP is
block-diagonal with exactly 8 blocks of 96 -> core i owns block i end-to-end).

Per-core pipeline (all matmuls bf16, fp32 PSUM). Layouts are chosen so that
every matmul rhs is contiguous-innermost and every PSUM eviction writes
contiguous (or long-run) destinations; evictions are split across DVE+ACT.

  S1  rfft over W:   lhsT=F1 [w,128]=[cos(65)|-sin(63)], rhs=xb [w, c, h]
                     4c-chunks -> psum [kwstack,(4c,128h)] -> t1 [kw, c, h]
  T1  DMA-xbar transpose (split on 2 engines): t1 -> t2 [h, c, kwstack]
  S2  DFT over H (data stationary): per kw: lhsT=t2[:, :, kw*] [h, c],
                     rhs=F2a/F2b [h, 256] -> psum [c, (khr|khi)] -> specw
  MLP1 (rhs mode):   lhsT=w1* [96,97] (col 96 zero, b1 row 96 = 1 ->
                     o1 row 96 == 1, the fused bias row for MLP2),
                     relu+b1 evict -> o1w [97, ri, kw, kh]
  MLP2 (data stationary): per kw: lhsT=o1w slices [97, kh],
                     rhs=[w2r|w2i ; b2r|b2i] / [-w2i|w2r ; 0]
                     -> psum [kh, (kw, cr|ci)]
  softshrink:        relu(v-l) + min(v+l, 0) -> o2 [kh, ri, c, kw]
  S4  iDFT over H (rhs mode): lhsT=Gc/Gs/-Gs, rhs=o2 4c-chunks (65-el runs)
                     -> psum [h, (4c, kw)] -> ubuf [h, c, kwstack]
                     (ui bins 0,64 dropped)
  T3  DMA-xbar transpose (split): ubuf -> s5rhs [kwstack, c, h]
  S5  irfft over W:  lhsT=Ainv [kstack, w], rhs=s5rhs + t1 (residual in
                     spectrum: irfft_W(rfft_W(x)) == x), 4c-chunks
                     -> psum [w, (4c, 128h)] -> out [w, c, h] f32

Host passes x pre-transposed to [B, W, 96, H] and un-transposes the
[B, W, 96, H] output, so all DMAs are fully contiguous per partition.
"""
import numpy as np
import ml_dtypes

B, H, W, C = 4, 128, 128, 768
NB, BL = 8, 96
WF = 65
LAMBD = 0.01
BF16 = ml_dtypes.bfloat16

_CACHE = {}


def _make_consts():
    w = np.arange(W, dtype=np.float64)[:, None]
    k = np.arange(WF, dtype=np.float64)[None, :]
    th = 2 * np.pi * w * k / W
    s = 1.0 / np.sqrt(W)
    f1 = np.concatenate([np.cos(th) * s, -np.sin(th[:, 1:64]) * s], axis=1)

    h = np.arange(H, dtype=np.float64)[:, None]
    kh = np.arange(H, dtype=np.float64)[None, :]
    th2 = 2 * np.pi * h * kh / H
    c2 = np.cos(th2) / np.sqrt(H)
    s2 = np.sin(th2) / np.sqrt(H)
    f2a = np.concatenate([c2, -s2], axis=1)   # rhs when lhsT = t_r
    f2b = np.concatenate([s2, c2], axis=1)    # rhs when lhsT = t_i

    gc = (np.cos(th2) / np.sqrt(H)).T         # [kh, h]
    gs = (np.sin(th2) / np.sqrt(H)).T

    kk = np.arange(WF, dtype=np.float64)[:, None]
    ww = np.arange(W, dtype=np.float64)[None, :]
    th3 = 2 * np.pi * kk * ww / W
    beta = np.full((WF, 1), 2.0); beta[0] = 1.0; beta[64] = 1.0
    ac = beta * np.cos(th3) / np.sqrt(W)
    asn = -2.0 * np.sin(th3[1:64]) / np.sqrt(W)
    ainv = np.concatenate([ac, asn], axis=0)

    cast = lambda a: np.ascontiguousarray(a).astype(BF16)
    return dict(f1=cast(f1), f2a=cast(f2a), f2b=cast(f2b),
                gc=cast(gc), gs=cast(gs), gsn=cast(-gs), ainv=cast(ainv))


def _groups():
    gs = [list(range(i, i + 8)) for i in range(0, 64, 8)]
    gs.append([64])
    return gs


def _build():
    from contextlib import ExitStack
    from concourse import bacc, mybir, tile

    dt = mybir.dt
    nc = bacc.Bacc("TRN2", target_bir_lowering=False, debug=False, num_devices=8)

    x_d = nc.dram_tensor("x", [B, W, BL, H], dt.bfloat16, kind="ExternalInput")
    f1_d = nc.dram_tensor("f1", [128, 128], dt.bfloat16, kind="ExternalInput")
    f2a_d = nc.dram_tensor("f2a", [128, 256], dt.bfloat16, kind="ExternalInput")
    f2b_d = nc.dram_tensor("f2b", [128, 256], dt.bfloat16, kind="ExternalInput")
    gc_d = nc.dram_tensor("gc", [128, 128], dt.bfloat16, kind="ExternalInput")
    gs_d = nc.dram_tensor("gs", [128, 128], dt.bfloat16, kind="ExternalInput")
    gsn_d = nc.dram_tensor("gsn", [128, 128], dt.bfloat16, kind="ExternalInput")
    ainv_d = nc.dram_tensor("ainv", [128, 128], dt.bfloat16, kind="ExternalInput")
    w1r_d = nc.dram_tensor("w1r", [BL, 97], dt.bfloat16, kind="ExternalInput")
    w1i_d = nc.dram_tensor("w1i", [BL, 97], dt.bfloat16, kind="ExternalInput")
    w1in_d = nc.dram_tensor("w1in", [BL, 97], dt.bfloat16, kind="ExternalInput")
    w2a_d = nc.dram_tensor("w2a", [97, 192], dt.bfloat16, kind="ExternalInput")
    w2b_d = nc.dram_tensor("w2b", [97, 192], dt.bfloat16, kind="ExternalInput")
    b1r_d = nc.dram_tensor("b1r", [97, 1], dt.float32, kind="ExternalInput")
    b1i_d = nc.dram_tensor("b1i", [97, 1], dt.float32, kind="ExternalInput")
    out_d = nc.dram_tensor("out", [B, W, BL, H], dt.float32, kind="ExternalOutput")

    Relu = mybir.ActivationFunctionType.Relu
    Ident = mybir.ActivationFunctionType.Identity
    ADD = mybir.AluOpType.add
    MAX = mybir.AluOpType.max
    MIN = mybir.AluOpType.min

    with tile.TileContext(nc) as tc, ExitStack() as ctx:
        cp = ctx.enter_context(tc.tile_pool(name="const", bufs=1))
        xp = ctx.enter_context(tc.tile_pool(name="xb", bufs=1))
        t1p = ctx.enter_context(tc.tile_pool(name="t1", bufs=2))
        t2p = ctx.enter_context(tc.tile_pool(name="t2", bufs=1))
        sw = ctx.enter_context(tc.tile_pool(name="specw", bufs=2))
        o1p = ctx.enter_context(tc.tile_pool(name="o1w", bufs=2))
        o2p = ctx.enter_context(tc.tile_pool(name="o2w", bufs=1))
        tap = ctx.enter_context(tc.tile_pool(name="tmpa", bufs=2))
        tbp = ctx.enter_context(tc.tile_pool(name="tmpb", bufs=2))
        up = ctx.enter_context(tc.tile_pool(name="ubuf", bufs=1))
        s5p = ctx.enter_context(tc.tile_pool(name="s5rhs", bufs=1))
        ocp = ctx.enter_context(tc.tile_pool(name="outc", bufs=2))
        psm = ctx.enter_context(tc.tile_pool(name="psmain", bufs=4, space="PSUM"))
        ps2p = ctx.enter_context(tc.tile_pool(name="ps2", bufs=2, space="PSUM"))
        pm2p = ctx.enter_context(tc.tile_pool(name="psm2", bufs=2, space="PSUM"))

        def cload(dram, shape, dtype=dt.bfloat16):
            t = cp.tile(shape, dtype, tag=f"c_{dram.name}")
            nc.sync.dma_start(t[:], dram[:])
            return t

        f1 = cload(f1_d, [128, 128]); f2a = cload(f2a_d, [128, 256])
        f2b = cload(f2b_d, [128, 256]); gc = cload(gc_d, [128, 128])
        gs = cload(gs_d, [128, 128]); gsn = cload(gsn_d, [128, 128])
        ainv = cload(ainv_d, [128, 128])
        w1r = cload(w1r_d, [BL, 97]); w1i = cload(w1i_d, [BL, 97])
        w1in = cload(w1in_d, [BL, 97])
        w2a = cload(w2a_d, [97, 192]); w2b = cload(w2b_d, [97, 192])
        b1r = cload(b1r_d, [97, 1], dt.float32)
        b1i = cload(b1i_d, [97, 1], dt.float32)
        lamneg = cp.tile([128, 1], dt.float32, tag="c_lamneg")
        nc.gpsimd.memset(lamneg[:], -LAMBD)
        zbias = cp.tile([128, 1], dt.float32, tag="c_zbias")
        nc.gpsimd.memset(zbias[:], 0.0)

        GROUPS = _groups()

        _flip = [0]

        def split_evict(dst_dve, src_dve, dst_act, src_act, act_bias=None,
                        act_func=None):
            # whole-tile eviction, alternating engines 3:2 DVE:ACT (two
            # engines reading halves of one PSUM tile raced on hardware)
            _flip[0] ^= 1
            if _flip[0]:
                nc.vector.tensor_copy(dst_dve, src_dve)
                nc.vector.tensor_copy(dst_act, src_act)
            else:
                p = src_dve.shape[0]
                nc.scalar.activation(dst_dve, src_dve, Ident, bias=zbias[0:p, :])
                nc.scalar.activation(dst_act, src_act, Ident, bias=zbias[0:p, :])

        prev = None  # (t1, s5r, b) of previous batch, for interleaved S5

        def emit_s5_group(st, gi):
            # 3 S5 chunks (12 c-columns) + 1 out-DMA per kw-group slot
            if st is None or gi >= 8:
                return
            t1q, s5q, bq = st
            co = gi * 12
            oc = ocp.tile([128, 12, 128], dt.float32, tag="oc")  # [w, c, h]
            for ci in range(co, co + 12, 4):
                ps5 = psm.tile([128, 4, 128], dt.float32, tag="ps")
                nc.tensor.matmul(ps5[:], ainv[:], s5q[:, ci:ci + 4, :],
                                 start=True, stop=False)
                nc.tensor.matmul(ps5[:], ainv[:], t1q[:, ci:ci + 4, :],
                                 start=False, stop=True)
                split_evict(oc[:, ci - co:ci - co + 2, :], ps5[:, 0:2, :],
                            oc[:, ci - co + 2:ci - co + 4, :], ps5[:, 2:4, :])
            nc.sync.dma_start(out_d[bq, :, co:co + 12, :], oc[:])

        def emit_front(b):
            # input DMA + S1 + T1 for batch b
            xb = xp.tile([128, BL, 128], dt.bfloat16, tag="xb")  # [w, c, h]
            nc.sync.dma_start(xb[:], x_d[b])
            t1 = t1p.tile([128, BL, 128], dt.bfloat16, tag="t1")  # [kw, c, h]
            for ci in range(0, BL, 4):
                ps = psm.tile([128, 4, 128], dt.float32, tag="ps")
                nc.tensor.matmul(ps[:], f1[:], xb[:, ci:ci + 4, :],
                                 start=True, stop=True)
                split_evict(t1[:, ci:ci + 2, :], ps[:, 0:2, :],
                            t1[:, ci + 2:ci + 4, :], ps[:, 2:4, :])
            t2 = t2p.tile([128, BL, 128], dt.bfloat16, tag="t2")  # [h, c, kwstack]
            nc.sync.dma_start_transpose(t2[:, 0:48, :], t1[:, 0:48, :])
            nc.scalar.dma_start_transpose(t2[:, 48:BL, :], t1[:, 48:BL, :])
            return t1, t2

        front = emit_front(0)

        for b in range(B + 1):
            if b == B:
                for gi in range(8):
                    emit_s5_group(prev, gi)
                break
            t1, t2 = front

            # ---- middle section per kw-group, interleaved with S5(b-1)
            o2 = o2p.tile([128, 2, BL, WF], dt.bfloat16, tag="o2")  # [kh,ri,c,kw]
            scr = up.tile([128, WF, 2, BL], dt.bfloat16, tag="ub")  # [kh,kw,ri,c]
            for gi, grp in enumerate(GROUPS):
                g0, gl = grp[0], len(grp)
                spec = sw.tile([BL, 8, 2, 128], dt.bfloat16, tag="spec")
                for j0 in range(0, gl, 2):
                    jl = min(2, gl - j0)
                    ps2 = ps2p.tile([BL, 2, 2, 128], dt.float32, tag="ps2")
                    for j in range(j0, j0 + jl):
                        kw = g0 + j
                        edge = kw in (0, 64)
                        nc.tensor.matmul(ps2[:, j - j0, :, :], t2[:, :, kw],
                                         f2a[:], start=True, stop=edge)
                        if not edge:
                            nc.tensor.matmul(ps2[:, j - j0, :, :],
                                             t2[:, :, 64 + kw], f2b[:],
                                             start=False, stop=True)
                    split_evict(spec[:, j0:j0 + jl, 0, :], ps2[:, 0:jl, 0, :],
                                spec[:, j0:j0 + jl, 1, :], ps2[:, 0:jl, 1, :])
                # MLP1 (M=97: row 96 == 1.0 bias row)
                o1 = o1p.tile([97, 2, 8, 128], dt.bfloat16, tag="o1")
                for c0 in range(0, gl, 4):
                    cl = min(4, gl - c0)
                    xr = spec[:, c0:c0 + cl, 0, :]
                    xi = spec[:, c0:c0 + cl, 1, :]
                    pr = psm.tile([97, 4, 128], dt.float32, tag="ps")
                    nc.tensor.matmul(pr[:, 0:cl, :], w1r[:], xr, start=True, stop=False)
                    nc.tensor.matmul(pr[:, 0:cl, :], w1in[:], xi, start=False, stop=True)
                    pi = psm.tile([97, 4, 128], dt.float32, tag="ps")
                    nc.tensor.matmul(pi[:, 0:cl, :], w1i[:], xr, start=True, stop=False)
                    nc.tensor.matmul(pi[:, 0:cl, :], w1r[:], xi, start=False, stop=True)
                    nc.scalar.activation(o1[:, 0, c0:c0 + cl, :], pr[:, 0:cl, :],
                                         Relu, bias=b1r[:])
                    nc.vector.tensor_scalar(o1[:, 1, c0:c0 + cl, :],
                                            pi[:, 0:cl, :], b1i[:], 0.0,
                                            ADD, MAX)
                # MLP2 (bias via o1 ones-row) + softshrink -> scr [kh, kw, ri, c]
                for j0 in range(0, gl, 2):
                    jl = min(2, gl - j0)
                    pm = pm2p.tile([128, 2, 2, BL], dt.float32, tag="pm2")
                    for j in range(j0, j0 + jl):
                        nc.tensor.matmul(pm[:, j - j0, :, :], o1[:, 0, j, :],
                                         w2a[:], start=True, stop=False)
                        nc.tensor.matmul(pm[:, j - j0, :, :], o1[:, 1, j, :],
                                         w2b[:], start=False, stop=True)
                    ta = tap.tile([128, 2, 2, BL], dt.bfloat16, tag="ta")
                    tb = tbp.tile([128, 2, 2, BL], dt.bfloat16, tag="tb")
                    nc.scalar.activation(ta[:, 0:jl], pm[:, 0:jl], Relu,
                                         bias=lamneg[:])
                    nc.vector.tensor_scalar(tb[:, 0:jl], pm[:, 0:jl],
                                            LAMBD, 0.0, ADD, MIN)
                    nc.gpsimd.tensor_tensor(scr[:, g0 + j0:g0 + j0 + jl, :, :],
                                            ta[:, 0:jl], tb[:, 0:jl], ADD)
                # interleave previous batch's S5 work
                emit_s5_group(prev, gi)

            # ---- front of b+1 overlaps b's tail (reorder/S4/T3)
            if b + 1 < B:
                front = emit_front(b + 1)

            # ---- reorder scratch [kh,kw,ri,c] -> o2 [kh,ri,c,kw]
            for ri in range(2):
                for qi, q in enumerate(range(0, BL, 24)):
                    src = scr[:, :, ri, q:q + 24].rearrange("k w c -> k c w")
                    eng = (nc.gpsimd, nc.vector, nc.gpsimd, nc.scalar)[qi]
                    if eng is nc.scalar:
                        nc.scalar.activation(o2[:, ri, q:q + 24, :], src,
                                             Ident, bias=zbias[:])
                    else:
                        eng.tensor_copy(o2[:, ri, q:q + 24, :], src)
            # ---- S4: 4c chunks; rhs 65-el runs; evictions 65/63-el runs
            ub = up.tile([128, BL, 128], dt.bfloat16, tag="ub")  # [h, c, kwstack]
            for ci in range(0, BL, 4):
                o2r = o2[:, 0, ci:ci + 4, :]
                o2i = o2[:, 1, ci:ci + 4, :]
                pu = psm.tile([128, 4, WF], dt.float32, tag="ps")
                nc.tensor.matmul(pu[:], gc[:], o2r, start=True, stop=False)
                nc.tensor.matmul(pu[:], gsn[:], o2i, start=False, stop=True)
                split_evict(ub[:, ci:ci + 2, 0:WF], pu[:, 0:2, :],
                            ub[:, ci + 2:ci + 4, 0:WF], pu[:, 2:4, :])
                pv = psm.tile([128, 4, 64], dt.float32, tag="ps")
                nc.tensor.matmul(pv[:], gs[:], o2r[:, :, 0:64],
                                 start=True, stop=False)
                nc.tensor.matmul(pv[:], gc[:], o2i[:, :, 0:64],
                                 start=False, stop=True)
                split_evict(ub[:, ci:ci + 2, WF:128], pv[:, 0:2, 1:64],
                            ub[:, ci + 2:ci + 4, WF:128], pv[:, 2:4, 1:64])

            # ---- T3 (halves)
            s5r = s5p.tile([128, BL, 128], dt.bfloat16, tag="s5r")  # [kstack, c, h]
            nc.sync.dma_start_transpose(s5r[:, 0:48, :], ub[:, 0:48, :])
            nc.scalar.dma_start_transpose(s5r[:, 48:BL, :], ub[:, 48:BL, :])
            prev = (t1, s5r, b)

    nc.compile()
    return nc


def get_nc():
    if "nc" not in _CACHE:
        _CACHE["nc"] = _build()
    return _CACHE["nc"]


def make_in_maps(x, w1, b1, w2, b2):
    consts = _make_consts()
    f32 = np.float32
    in_maps = []
    for i in range(NB):
        sl = slice(BL * i, BL * (i + 1))
        m = dict(consts)
        m["x"] = np.ascontiguousarray(
            x[..., sl].transpose(0, 2, 3, 1)).astype(BF16)  # [B, W, 96, H]
        w1p = np.zeros((2, BL, 97), np.float64)
        w1p[:, :, 0:BL] = w1[:, i]
        m["w1r"] = w1p[0].astype(BF16)
        m["w1i"] = w1p[1].astype(BF16)
        m["w1in"] = (-w1p[1]).astype(BF16)
        w2a = np.zeros((97, 192), np.float64)
        w2a[0:BL, 0:BL] = w2[0, i]; w2a[0:BL, BL:] = w2[1, i]
        w2a[BL, 0:BL] = b2[0, i]; w2a[BL, BL:] = b2[1, i]
        w2bp = np.zeros((97, 192), np.float64)
        w2bp[0:BL, 0:BL] = -w2[1, i]; w2bp[0:BL, BL:] = w2[0, i]
        m["w2a"] = w2a.astype(BF16)
        m["w2b"] = w2bp.astype(BF16)
        b1p = np.zeros((2, 97, 1), np.float64)
        b1p[:, 0:BL, 0] = b1[:, i]
        b1p[0, BL, 0] = 1.0  # ones row for fused MLP2 bias
        m["b1r"] = b1p[0].astype(f32)
        m["b1i"] = b1p[1].astype(f32)
        in_maps.append(m)
    return in_maps


def kernel(x, w1, b1, w2, b2):
    from concourse.bass_utils import run_bass_kernel_spmd
    nc = get_nc()
    in_maps = make_in_maps(np.asarray(x), np.asarray(w1), np.asarray(b1),
                           np.asarray(w2), np.asarray(b2))
    res = run_bass_kernel_spmd(nc, in_maps, core_ids=list(range(NB)))
    out = np.concatenate(
        [res.results[i]["out"].transpose(0, 3, 1, 2) for i in range(NB)],
        axis=-1)
    return out.astype(np.float32)
